# revision 28
# baseline (speedup 1.0000x reference)
"""Trainium2 Bass kernel for nn_MessagePassing (gnn_message_passing).

Self-contained: takes full (unsharded) numpy inputs, shards batch*rounds
across 8 NeuronCores, runs a Bass/Tile kernel per core, gathers the full
output.

Math (per (b,r) group, all biases included):
  q      = Wq @ ques + bq                       [H]
  edges  = W1a @ on + W1b @ adj + b1            [H, N*E]  (on broadcast over E)
  a      = softmax_E(We @ (q*edges) + be)
  edges2 = a * edges
  t      = W2a @ adj + W2b @ edges2 + b2
  b      = softmax_E(Wv @ (q*t) + bv)
  out    = sum_E b * (Wadj @ adj + badj)        [H, N]

Design (fp8 DoubleRow + algebraic restructure), HW 325us vs 365us
bf16 baseline, rel err 6.0e-3 (gate 2e-2):
  * Heavy GEMMs (A, B, F) run in fp8 e4m3 with DoubleRow perf mode
    (K=256 per instruction, 2x PE throughput). Stage H (Wadj@adj, the
    only path that touches the output directly) stays bf16.
  * q is folded into the PSUM->SBUF copy scales (per-partition ACT
    scale), so We/Wv are static host-quantized fp8 - no per-group
    weight folds for stages B/F.
  * Stage E (t) is eliminated: t only feeds the b-logits, so
    logits_b = M1 @ adj + M2 @ edges2 + (Wv @ (q*b2)) + bv with
    M1 = Wv diag(q) W2a, M2 = Wv diag(q) W2b folded per group on the
    PE (fp8 DoubleRows over [512,512] - ~7k cycles).
  * badj is folded into stage H's contraction via a ones-row
    (out = recb * sum_E expb*(adj'+badj) works because sum_E b = 1).
  * softmax sums in f32 (DVE reduces run 1x regardless; f32 is free
    accuracy). NOTE: vector.reciprocal with a bf16 input silently
    corrupts results on HW - reciprocal inputs must be f32.
  * all per-group scalar work (q chain, ontT, weight folds, M1/M2,
    bvx) is hoisted into a one-time prologue (ques/on are tiny); the
    steady-state loop is only A/B/CD/F'/G/H/I, interleaved at m-chunk
    granularity, with 64 PSUM allocs per iteration (8-bank aligned).

Layout on device: hidden channels on partitions (4 chunks of 128), tokens
(node*E+e) on the free dim, so softmax over E is a free-dim segment reduce.
fp8 contraction operands are plane-packed: [128, nplanes, F] where plane p
holds contraction rows p*128..(p+1)*128-1.
"""

import os
import sys

for _p in ("/opt/trn_rl_repo", "/root/.axon_site/_ro/trn_rl_repo",
           "/root/.axon_site/_ro/pypackages"):
    if _p not in sys.path and os.path.isdir(_p):
        sys.path.append(_p)

import contextlib
import ctypes
import types

import ml_dtypes
import numpy as np

import concourse.bass as bass
import concourse.tile as tile
from concourse import mybir

BF = mybir.dt.bfloat16
F32 = mybir.dt.float32
F8 = mybir.dt.float8e4
AX = mybir.AxisListType
ALU = mybir.AluOpType
ACTF = mybir.ActivationFunctionType
DR = mybir.MatmulPerfMode.DoubleRow

B, R, N, E, D, H = 4, 10, 80, 20, 300, 512
BR = B * R              # 40 (b,r) groups
NCORES = 8
G = BR // NCORES        # 5 groups per core
TOK = N * E             # 1600 tokens per group
NT = 4                  # token tiles per group
T = TOK // NT           # 400 tokens per tile
DRAG = D - 256          # 44 ragged contraction rows of the D=300 dim
KX = N + 1 + DRAG       # 125: [smat | ones | adj-ragged] packed stage-A chunk

KD = [(0, 128), (128, 256), (256, 300)]               # D=300 contraction chunks
KH = [(0, 128), (128, 256), (256, 384), (384, 512)]   # H=512 contraction chunks
KD1 = [(0, 128), (128, 256), (256, 301)]              # D+1 (wadj+badj row)
MS = [(0, 128), (128, 256), (256, 384), (384, 512)]   # output chunks

# ---- scale constants (see emulate.py for the validated algebra) ----
SW1 = 32.0    # W1 stationary scale (w1a32/w1b8/w1brag32/b1row32)
SE8 = 4.0     # edges8 = SE8*q*edges       (A-copy scale = q*SE8/SW1)
SWE = 64.0    # we8 = SWE*We               (B-exp scale = 1/(SWE*SE8))
SE2 = 16.0    # edges2_8 = SE2*a*edges     (recaq = SE2/(SE8*q*suma))
SWV = 64.0    # wv8 = SWV*Wv
CF = 16.0     # w2aq8/w2bq8 = CF*q*W2xT
CM1 = 8192.0  # m1t8 = CM1*M1T             (copy scale = CM1/(CF*SWV) = 8)
CM2 = 512.0   # m2t8 = CM2*M2T (CM2*SE2 == CM1 so the F psum scales match)
RECQ_CLAMP = 1e4

_MAXW = 1  # this walrus build allows a single semaphore wait per instruction


def _split_multi_waits(nc):
    """Walrus here rejects instructions with >1 sem wait; hoist extra waits
    onto same-engine NoOps inserted just before the instruction."""
    ctr = 0
    for fn in nc.m.functions:
        for bb in fn.blocks:
            new = []
            for inst in bb.instructions:
                si = inst.sync_info
                if si is not None:
                    waits = list(si.on_wait)
                    if len(waits) > _MAXW:
                        for i in range(0, len(waits) - _MAXW, _MAXW):
                            ctr += 1
                            nop = mybir.InstNoOp(name=f"wsplit-{ctr}")
                            nop.engine = inst.engine
                            nop.sync_info = mybir.SyncInfo(
                                on_wait=waits[i : i + _MAXW], on_update=[]
                            )
                            new.append(nop)
                        si.on_wait = waits[len(waits) - _MAXW :]
                new.append(inst)
            bb.instructions = new
    return ctr


def _patch_ldw_dedupe():
    """The bass pipeline splits every matmul into Ldweights + Matmult.
    Consecutive matmuls that share the stationary operand then reload the
    same weights. Drop the redundant Ldweights at the BIR-JSON level
    (walrus's own --enable-ldw-opt rejects explicit Ldweights)."""
    import orjson

    import concourse.bass2jax as b2j
    import concourse.bass_utils as bu

    if getattr(bu, "_ldw_dedupe_patched", False):
        return
    orig = bu.compile_bir_kernel

    def _dedupe(bir_json):
        d = orjson.loads(bir_json)
        removed = 0
        nopctr = 0
        for fn in d.get("functions", []):
            stack = list(fn.get("blocks", []))
            while stack:
                blk = stack.pop()
                stack.extend(blk.get("blocks", []))
                insts = blk.get("instructions", [])
                out = []
                last_key = None
                for i in insts:
                    op = i.get("opcode")
                    if op == "Ldweights":
                        key = orjson.dumps(
                            [
                                i.get("ins"),
                                i.get("perf_mode"),
                                i.get("tile_position"),
                                i.get("tile_size"),
                                i.get("is_transpose"),
                            ]
                        )
                        si = i.get("sync_info") or {}
                        if key == last_key and not si.get("on_update"):
                            w = si.get("on_wait") or []
                            if w:
                                nopctr += 1
                                out.append(
                                    {
                                        "name": f"ldwkeep-{nopctr}",
                                        "opcode": "NoOp",
                                        "engine": i.get("engine", "PE"),
                                        "ins": [],
                                        "outs": [],
                                        "sync_info": {
                                            "on_wait": w,
                                            "on_update": [],
                                        },
                                    }
                                )
                            removed += 1
                            continue
                        last_key = key
                    elif op == "Matmult":
                        if i.get("is_transpose") or i.get("ldweights"):
                            last_key = None
                    out.append(i)
                blk["instructions"] = out
        if os.environ.get("KERNEL_DEBUG"):
            print(f"ldw dedupe: removed {removed}", file=sys.stderr)
        return orjson.dumps(d)

    def compile_bir_kernel(bir_json, tmpdir, neff_name="file.neff"):
        try:
            bir_json = _dedupe(bir_json)
        except Exception as e:  # pragma: no cover - safety net
            print(f"ldw dedupe skipped: {e}", file=sys.stderr)
        return orig(bir_json, tmpdir, neff_name=neff_name)

    bu.compile_bir_kernel = compile_bir_kernel
    b2j.compile_bir_kernel = compile_bir_kernel
    bu._ldw_dedupe_patched = True


def _install_ntff_hook():
    """Provide antenv.axon_hooks (missing in this image) so that
    run_bass_kernel_spmd(trace=True) can profile via libaxon_pjrt."""
    if "antenv.axon_hooks" in sys.modules:
        return

    def _mk(so_path):
        try:
            lib = ctypes.CDLL(so_path)
        except OSError:
            return None
        if not hasattr(lib, "axon_start_nrt_profile"):
            return None
        lib.axon_start_nrt_profile.argtypes = [
            ctypes.POINTER(ctypes.c_int64),
            ctypes.c_size_t,
        ]
        lib.axon_start_nrt_profile.restype = ctypes.c_int64
        lib.axon_stop_nrt_profile.argtypes = [ctypes.c_char_p]
        lib.axon_stop_nrt_profile.restype = ctypes.c_int64

        @contextlib.contextmanager
        def _hook(output_dir, device_ids):
            import jax

            jax.devices()
            if device_ids:
                ids = (ctypes.c_int64 * len(device_ids))(*device_ids)
                rc = lib.axon_start_nrt_profile(ids, len(device_ids))
            else:
                rc = lib.axon_start_nrt_profile(None, 0)
            if rc != 0:
                raise RuntimeError(f"axon_start_nrt_profile rc={rc}")
            try:
                yield
            finally:
                n = lib.axon_stop_nrt_profile(str(output_dir).encode())
                print(f"ntff profile: {n} file(s) -> {output_dir}", file=sys.stderr)

        return _hook

    hook = _mk("/opt/axon/libaxon_pjrt.so")
    mod = types.ModuleType("antenv.axon_hooks")
    mod.get_axon_ntff_profile_hook = lambda: hook
    try:
        import antenv

        antenv.axon_hooks = mod
    except ImportError:
        pass
    sys.modules["antenv.axon_hooks"] = mod

    import concourse.bass_utils as bass_utils

    bass_utils.upload_artifacts = lambda tmpdir: f"local://{tmpdir}"


def _re3(ap):
    """[128, n*E] -> [128, n, E] view."""
    return ap.rearrange("p (n e) -> p n e", e=E)


def build_program():
    nc = bass.Bass()

    # --- per-group data ---
    adjT = nc.declare_dram_parameter("adjT", [G, D + 1, TOK], BF, isOutput=False)
    adj8_d = nc.declare_dram_parameter("adj8", [G, 128, 2 * TOK], F8, isOutput=False)
    onT = nc.declare_dram_parameter("onT", [G, D, N], BF, isOutput=False)
    quesT = nc.declare_dram_parameter("quesT", [G, 128, 4], BF, isOutput=False)
    # --- static weights ---
    w1a32_d = nc.declare_dram_parameter("w1a32", [D, H], BF, isOutput=False)
    w1b8_d = nc.declare_dram_parameter("w1b8", [128, 2 * H], F8, isOutput=False)
    w1brag32_d = nc.declare_dram_parameter("w1brag32", [DRAG, H], BF, isOutput=False)
    b1row32_d = nc.declare_dram_parameter("b1row32", [1, H], BF, isOutput=False)
    smat_d = nc.declare_dram_parameter("smat", [N + 1, TOK], BF, isOutput=False)
    wq_d = nc.declare_dram_parameter("wq", [H, H], BF, isOutput=False)
    we8_d = nc.declare_dram_parameter("we8", [128, 4 * H], F8, isOutput=False)
    wv8_d = nc.declare_dram_parameter("wv8", [128, 4 * H], F8, isOutput=False)
    w2aT_d = nc.declare_dram_parameter("w2aT", [128, 4 * D], BF, isOutput=False)
    w2bT_d = nc.declare_dram_parameter("w2bT", [128, 4 * H], BF, isOutput=False)
    wadjx_d = nc.declare_dram_parameter("wadjx", [D + 1, H], BF, isOutput=False)
    # biases packed [128, 4] (column j = channels j*128..j*128+127)
    bq_d = nc.declare_dram_parameter("bq", [128, 4], F32, isOutput=False)
    be_d = nc.declare_dram_parameter("be", [128, 4], F32, isOutput=False)
    bv_d = nc.declare_dram_parameter("bv", [128, 4], F32, isOutput=False)
    b2c_d = nc.declare_dram_parameter("b2c", [128, 4], BF, isOutput=False)

    outT = nc.declare_dram_parameter("outT", [G, 4, 128, N], F32, isOutput=True)

    def tsl(t):
        return slice(t * T, (t + 1) * T)

    ctx0 = nc.allow_low_precision("softmax sums kept in bf16 deliberately")
    ctx0.__enter__()
    with tile.TileContext(nc) as tc, contextlib.ExitStack() as ctx:
        wpool = ctx.enter_context(tc.tile_pool(name="weights", bufs=1))
        gpool = ctx.enter_context(tc.tile_pool(name="group", bufs=2))
        gpool3 = ctx.enter_context(tc.tile_pool(name="group3", bufs=3))
        spool = ctx.enter_context(tc.tile_pool(name="small", bufs=2))
        spool3 = ctx.enter_context(tc.tile_pool(name="small3", bufs=3))
        ppool = ctx.enter_context(tc.tile_pool(name="pergroup", bufs=G))
        pspool = ctx.enter_context(tc.tile_pool(name="ps", bufs=8, space="PSUM"))

        # PE warmup: keep the HAM clock-gate at 8/8 through the startup
        # DMA wait so the first real matmuls run at 2.4 GHz.
        wu_sb = wpool.tile([128, 512], BF, tag="wu", name="wu")
        nc.vector.memset(wu_sb[:], 0.0)
        wu_ps = pspool.tile([128, T], F32, tag="ps", name="wups")
        for i in range(85):
            nc.tensor.matmul(
                wu_ps[:], wu_sb[:, :128], wu_sb[:, :T], start=True, stop=True
            )

        def load_w(dram, shape, dt_, name):
            t_ = wpool.tile(shape, dt_, tag=name, name=name)
            nc.scalar.dma_start(out=t_[:], in_=dram[:, :])
            return t_

        def load_w_chunks(dram, chunks, name):
            tiles = []
            for ki, (k0, k1) in enumerate(chunks):
                t_ = wpool.tile(
                    [k1 - k0, H], BF, tag=f"{name}{ki}", name=f"{name}{ki}"
                )
                nc.scalar.dma_start(out=t_[:], in_=dram[k0:k1, :])
                tiles.append(t_)
            return tiles

        w1a32_sb = load_w_chunks(w1a32_d, KD, "w1a32")
        wq_sb = load_w_chunks(wq_d, KH, "wq")
        wadjx_sb = load_w_chunks(wadjx_d, KD1, "wadjx")
        w1b8_sb = load_w(w1b8_d, [128, 2, H], F8, "w1b8")
        we8_sb = load_w(we8_d, [128, 4, H], F8, "we8")
        wv8_sb = load_w(wv8_d, [128, 4, H], F8, "wv8")
        w2aT_sb = load_w(w2aT_d, [128, 4, D], BF, "w2aT")
        w2bT_sb = load_w(w2bT_d, [128, 4, H], BF, "w2bT")
        bq_sb = load_w(bq_d, [128, 4], F32, "bq")
        be_sb = load_w(be_d, [128, 4], F32, "be")
        bv_sb = load_w(bv_d, [128, 4], F32, "bv")
        b2c_sb = load_w(b2c_d, [128, 4], BF, "b2c")

        def pre_dma_small(g):
            """Prologue DMAs: ques/on + the static rows of w1x."""
            st = {}
            ques_sb = spool.tile([128, 4], BF, tag="ques", name=f"ques_{g}")
            nc.sync.dma_start(out=ques_sb[:], in_=quesT[g, :, :])
            on_sb = []
            for ki, (k0, k1) in enumerate(KD):
                t_ = spool.tile(
                    [k1 - k0, N], BF, tag=f"on{ki}", name=f"on{ki}_{g}"
                )
                nc.sync.dma_start(out=t_[:], in_=onT[g, k0:k1, :])
                on_sb.append(t_)
            w1x_sb = ppool.tile([KX, H], BF, tag="w1x", name=f"w1x_{g}")
            nc.sync.dma_start(out=w1x_sb[N : N + 1, :], in_=b1row32_d[:, :])
            nc.sync.dma_start(out=w1x_sb[N + 1 :, :], in_=w1brag32_d[:, :])
            st["ques"] = ques_sb
            st["on"] = on_sb
            st["w1x"] = w1x_sb
            return st

        def pre_dma_big(g, st):
            """Per-iteration DMAs: adjacency tensors."""
            adjx_sb = gpool3.tile([KX, TOK], BF, tag="adjx", name=f"adjx_{g}")
            nc.sync.dma_start(out=adjx_sb[: N + 1, :], in_=smat_d[:, :])
            nc.sync.dma_start(out=adjx_sb[N + 1 :, :], in_=adjT[g, 256:D, :])
            adj8_sb = gpool3.tile([128, 2, TOK], F8, tag="adj8", name=f"adj8_{g}")
            nc.sync.dma_start(out=adj8_sb[:], in_=adj8_d[g, :, :])
            adjc = []
            for ki, (k0, k1) in enumerate(KD1):
                t_ = gpool3.tile(
                    [k1 - k0, TOK], BF, tag=f"adj{ki}", name=f"adj{ki}_{g}"
                )
                nc.sync.dma_start(out=t_[:], in_=adjT[g, k0:k1, :])
                adjc.append(t_)
            st["adjx"] = adjx_sb
            st["adj8"] = adj8_sb
            st["adjc"] = adjc

        def pre_early(g, st):
            """q chain + ontT + ACT weight folds (no M matmuls yet)."""
            ques_sb = st["ques"]
            on_sb = st["on"]
            w1x_sb = st["w1x"]

            # q = Wq @ ques + bq  (f32, kept for copy scales only)
            q_ps = pspool.tile([128, 4], F32, tag="ps", name=f"qps_{g}")
            for m, (m0, m1) in enumerate(MS):
                for k in range(4):
                    nc.tensor.matmul(
                        q_ps[:, m : m + 1],
                        wq_sb[k][:, m0:m1],
                        ques_sb[:, k : k + 1],
                        start=(k == 0),
                        stop=(k == 3),
                    )
            q_sb = spool.tile([128, 4], F32, tag="q", name=f"q_{g}")
            for m in range(4):
                nc.scalar.activation(
                    out=q_sb[:, m : m + 1],
                    in_=q_ps[:, m : m + 1],
                    func=ACTF.Identity,
                    bias=bq_sb[:, m : m + 1],
                )
            qA_sb = ppool.tile([128, 4], F32, tag="qA", name=f"qA_{g}")
            nc.scalar.mul(qA_sb[:], q_sb[:], SE8 / SW1)
            qC_sb = spool.tile([128, 4], F32, tag="qC", name=f"qC_{g}")
            nc.scalar.mul(qC_sb[:], q_sb[:], CF)
            rq_sb = spool.tile([128, 4], F32, tag="rq", name=f"rq_{g}")
            nc.vector.reciprocal(rq_sb[:], q_sb[:])
            recq_sb = spool.tile([128, 4], F32, tag="recq", name=f"recq_{g}")
            nc.vector.tensor_scalar(
                out=recq_sb[:],
                in0=rq_sb[:],
                scalar1=RECQ_CLAMP,
                scalar2=-RECQ_CLAMP,
                op0=ALU.min,
                op1=ALU.max,
            )
            # recqf = recq*(SE2/SE8): folded scale for the softmax-a chain
            recqf_sb = ppool.tile([128, 4], F32, tag="recqf", name=f"recqf_{g}")
            nc.scalar.mul(recqf_sb[:], recq_sb[:], SE2 / SE8)
            st["qA"] = qA_sb
            st["qC"] = qC_sb
            st["recqf"] = recqf_sb

            # transposed on-term: ontT[n, c] = SW1 * sum_f on[f, n] W1a[f, c]
            ontT_ps = pspool.tile([N, H], F32, tag="ps", name=f"ontTps_{g}")
            for ki in range(3):
                nc.tensor.matmul(
                    ontT_ps[:],
                    on_sb[ki][:],
                    w1a32_sb[ki][:],
                    start=(ki == 0),
                    stop=(ki == 2),
                )
            nc.scalar.copy(out=w1x_sb[:N, :], in_=ontT_ps[:])

            # folds: w2aq8 = CF*q (.) w2aT ; w2bq8 = CF*q (.) w2bT   (ACT)
            w2aq8 = ppool.tile([128, 4, H], F8, tag="w2aq8", name=f"w2aq8_{g}")
            w2bq8 = spool3.tile([128, 4, H], F8, tag="w2bq8", name=f"w2bq8_{g}")
            for k in range(4):
                nc.scalar.activation(
                    out=w2aq8[:, k, :D], in_=w2aT_sb[:, k, :],
                    func=ACTF.Copy, scale=qC_sb[:, k : k + 1],
                )
                nc.scalar.activation(
                    out=w2bq8[:, k, :], in_=w2bT_sb[:, k, :],
                    func=ACTF.Copy, scale=qC_sb[:, k : k + 1],
                )
            st["w2aq8"] = w2aq8
            st["w2bq8"] = w2bq8

        def pre_late(g, st):
            """M1/M2 fold matmuls + fp8 copies + b2 bias fold."""
            qC_sb = st["qC"]
            w2aq8 = st["w2aq8"]
            w2bq8 = st["w2bq8"]

            m1t8 = ppool.tile([128, 2, H], F8, tag="m1t8", name=f"m1t8_{g}")
            m1trag = ppool.tile([DRAG, H], BF, tag="m1trag", name=f"m1trag_{g}")
            for dc, (d0, d1) in enumerate(KD):
                psM = pspool.tile([d1 - d0, H], F32, tag="ps", name=f"m1ps_{g}_{dc}")
                for i in range(2):
                    nc.tensor.matmul(
                        psM[:],
                        w2aq8[:, 2 * i : 2 * i + 2, d0:d1],
                        wv8_sb[:, 2 * i : 2 * i + 2, :],
                        start=(i == 0),
                        stop=(i == 1),
                        perf_mode=DR,
                    )
                if dc < 2:
                    nc.vector.tensor_scalar_mul(
                        m1t8[:, dc, :], psM[:], CM1 / (CF * SWV)
                    )
                else:
                    nc.vector.tensor_scalar_mul(
                        m1trag[:, :], psM[:], CM1 / (CF * SWV)
                    )
            m2t8a = ppool.tile([128, 2, H], F8, tag="m2t8a", name=f"m2t8a_{g}")
            m2t8b = ppool.tile([128, 2, H], F8, tag="m2t8b", name=f"m2t8b_{g}")
            for kc, (k0, k1) in enumerate(KH):
                psM = pspool.tile([128, H], F32, tag="ps", name=f"m2ps_{g}_{kc}")
                for i in range(2):
                    nc.tensor.matmul(
                        psM[:],
                        w2bq8[:, 2 * i : 2 * i + 2, k0:k1],
                        wv8_sb[:, 2 * i : 2 * i + 2, :],
                        start=(i == 0),
                        stop=(i == 1),
                        perf_mode=DR,
                    )
                dst = m2t8a if kc < 2 else m2t8b
                nc.vector.tensor_scalar_mul(
                    dst[:, kc % 2, :], psM[:], CM2 / (CF * SWV)
                )
            st["m1t8"] = m1t8
            st["m1trag"] = m1trag
            st["m2t8a"] = m2t8a
            st["m2t8b"] = m2t8b

            # b2 correction folded into the F-exp bias:
            # bvx = bv + Wv @ (q*b2) = bv + wv8^T @ qb2_8 / (SWV*CF)
            qb2_8 = spool.tile([128, 4], F8, tag="qb2", name=f"qb2_{g}")
            nc.vector.tensor_tensor(
                out=qb2_8[:], in0=qC_sb[:], in1=b2c_sb[:], op=ALU.mult
            )
            qb2_ps = pspool.tile([128, 4], F32, tag="ps", name=f"qb2ps_{g}")
            for m, (m0, m1) in enumerate(MS):
                for k in range(4):
                    nc.tensor.matmul(
                        qb2_ps[:, m : m + 1],
                        wv8_sb[:, k, m0:m1],
                        qb2_8[:, k : k + 1],
                        start=(k == 0),
                        stop=(k == 3),
                    )
            bvx_sb = ppool.tile([128, 4], F32, tag="bvx", name=f"bvx_{g}")
            for m in range(4):
                nc.scalar.activation(
                    out=bvx_sb[:, m : m + 1],
                    in_=qb2_ps[:, m : m + 1],
                    func=ACTF.Identity,
                    scale=1.0 / (SWV * CF),
                    bias=bv_sb[:, m : m + 1],
                )
            st["bvx"] = bvx_sb

        def emit_A(g, st, ms):
            """Stage A (edges psum + fp8 copy) for m-chunks in ms."""
            adj8_sb = st["adj8"]
            w1x_sb = st["w1x"]
            adjx_sb = st["adjx"]
            qA_sb = st["qA"]
            if "edges8" not in st:
                st["edges8"] = gpool.tile(
                    [128, 4, TOK], F8, tag="edges8", name=f"edges8_{g}"
                )
                st["expa"] = [
                    gpool.tile([128, TOK], BF, tag=f"expa{m}", name=f"expa{m}_{g}")
                    for m in range(4)
                ]
                st["e2"] = gpool.tile([128, 4, TOK], F8, tag="e2", name=f"e2_{g}")
            edges8 = st["edges8"]

            for m in ms:
                m0, m1 = MS[m]
                eps = [
                    pspool.tile([128, T], F32, tag="ps", name=f"eps_{g}_{m}_{t}")
                    for t in range(NT)
                ]
                for t in range(NT):
                    nc.tensor.matmul(
                        eps[t][:],
                        w1b8_sb[:, :, m0:m1],
                        adj8_sb[:, :, tsl(t)],
                        start=True,
                        stop=False,
                        perf_mode=DR,
                    )
                for t in range(NT):
                    nc.tensor.matmul(
                        eps[t][:],
                        w1x_sb[:, m0:m1],
                        adjx_sb[:, tsl(t)],
                        start=False,
                        stop=True,
                    )
                # A-copy (ACT): edges8 = psum * (q*SE8/SW1), fp8 out
                for t in range(NT):
                    nc.scalar.activation(
                        out=edges8[:, m, tsl(t)],
                        in_=eps[t][:],
                        func=ACTF.Copy,
                        scale=qA_sb[:, m : m + 1],
                    )

        def emit_B(g, st):
            """Stage B: expa = exp(psum/(SWE*SE8) + be)."""
            edges8 = st["edges8"]
            expa_sb = st["expa"]
            for m, (m0, m1) in enumerate(MS):
                lps = [
                    pspool.tile([128, T], F32, tag="ps", name=f"lps_{g}_{m}_{t}")
                    for t in range(NT)
                ]
                for i in range(2):
                    for t in range(NT):
                        nc.tensor.matmul(
                            lps[t][:],
                            we8_sb[:, 2 * i : 2 * i + 2, m0:m1],
                            edges8[:, 2 * i : 2 * i + 2, tsl(t)],
                            start=(i == 0),
                            stop=(i == 1),
                            perf_mode=DR,
                        )
                for t in range(NT):
                    nc.scalar.activation(
                        out=expa_sb[m][:, tsl(t)],
                        in_=lps[t][:],
                        func=ACTF.Exp,
                        scale=1.0 / (SWE * SE8),
                        bias=be_sb[:, m : m + 1],
                    )

        def emit_CD(g, st):
            """softmax-a chain + edges2 per m-chunk."""
            edges8 = st["edges8"]
            expa_sb = st["expa"]
            e2_8 = st["e2"]
            recqf_sb = st["recqf"]
            for m in range(4):
                suma = spool.tile([128, N], F32, tag=f"suma{m}", name=f"suma{m}_{g}")
                nc.vector.tensor_reduce(
                    suma[:], _re3(expa_sb[m][:]), axis=AX.X, op=ALU.add
                )
                rs = spool.tile([128, N], F32, tag=f"rs{m}", name=f"rs{m}_{g}")
                nc.vector.reciprocal(rs[:], suma[:])
                recaq = spool.tile(
                    [128, N], F32, tag=f"recaq{m}", name=f"recaq{m}_{g}"
                )
                nc.vector.tensor_scalar_mul(recaq[:], rs[:], recqf_sb[:, m : m + 1])
                # aN = expa * recaq (broadcast over E) in place   (Pool)
                nc.gpsimd.tensor_tensor(
                    out=_re3(expa_sb[m][:]),
                    in0=_re3(expa_sb[m][:]),
                    in1=recaq[:, :, None].broadcast_to((128, N, E)),
                    op=ALU.mult,
                )
                # edges2_8 = aN * edges8 (Pool, fp8 out)
                nc.gpsimd.tensor_tensor(
                    out=e2_8[:, m, :],
                    in0=expa_sb[m][:],
                    in1=edges8[:, m, :],
                    op=ALU.mult,
                )

        def emit_F(g, st, ms):
            """Stage F' (logits_b via M1/M2) + expb for m-chunks in ms."""
            adjc = st["adjc"]
            adj8_sb = st["adj8"]
            e2_8 = st["e2"]
            m1t8 = st["m1t8"]
            m1trag = st["m1trag"]
            m2t8a = st["m2t8a"]
            m2t8b = st["m2t8b"]
            bvx_sb = st["bvx"]
            if "expb" not in st:
                st["expb"] = [
                    gpool.tile([128, TOK], BF, tag=f"expb{m}", name=f"expb{m}_{g}")
                    for m in range(4)
                ]
            expb_sb = st["expb"]

            for m in ms:
                m0, m1 = MS[m]
                bps = [
                    pspool.tile([128, T], F32, tag="ps", name=f"bps_{g}_{m}_{t}")
                    for t in range(NT)
                ]
                for t in range(NT):
                    nc.tensor.matmul(
                        bps[t][:],
                        m1t8[:, :, m0:m1],
                        adj8_sb[:, :, tsl(t)],
                        start=True,
                        stop=False,
                        perf_mode=DR,
                    )
                for t in range(NT):
                    nc.tensor.matmul(
                        bps[t][:],
                        m1trag[:, m0:m1],
                        adjc[2][:DRAG, tsl(t)],
                        start=False,
                        stop=False,
                    )
                for t in range(NT):
                    nc.tensor.matmul(
                        bps[t][:],
                        m2t8a[:, :, m0:m1],
                        e2_8[:, 0:2, tsl(t)],
                        start=False,
                        stop=False,
                        perf_mode=DR,
                    )
                for t in range(NT):
                    nc.tensor.matmul(
                        bps[t][:],
                        m2t8b[:, :, m0:m1],
                        e2_8[:, 2:4, tsl(t)],
                        start=False,
                        stop=True,
                        perf_mode=DR,
                    )
                for t in range(NT):
                    nc.scalar.activation(
                        out=expb_sb[m][:, tsl(t)],
                        in_=bps[t][:],
                        func=ACTF.Exp,
                        scale=1.0 / CM1,
                        bias=bvx_sb[:, m : m + 1],
                    )

        def emit_G(g, st):
            expb_sb = st["expb"]
            sumb = spool.tile([128, 4, N], F32, tag="sumb", name=f"sumb_{g}")
            for m in range(4):
                nc.vector.tensor_reduce(
                    sumb[:, m, :], _re3(expb_sb[m][:]), axis=AX.X, op=ALU.add
                )
            recb = spool.tile([128, 4, N], F32, tag="recb", name=f"recb_{g}")
            nc.vector.reciprocal(recb[:], sumb[:])
            st["recb"] = recb
            st["s"] = spool.tile([128, 4, N], F32, tag="s", name=f"s_{g}")

        def emit_H(g, st, ms):
            """Stage H (bf16, badj via ones-row) for m-chunks in ms."""
            adjc = st["adjc"]
            expb_sb = st["expb"]
            for m in ms:
                m0, m1 = MS[m]
                aps = [
                    pspool.tile([128, T], F32, tag="ps", name=f"aps_{g}_{m}_{t}")
                    for t in range(NT)
                ]
                for ki in range(3):
                    for t in range(NT):
                        nc.tensor.matmul(
                            aps[t][:],
                            wadjx_sb[ki][:, m0:m1],
                            adjc[ki][:, tsl(t)],
                            start=(ki == 0),
                            stop=(ki == 2),
                        )
                # pre = psum * expb  (DVE - GPSIMD cannot access PSUM)
                for t in range(NT):
                    nc.vector.tensor_tensor(
                        out=expb_sb[m][:, tsl(t)],
                        in0=aps[t][:],
                        in1=expb_sb[m][:, tsl(t)],
                        op=ALU.mult,
                    )
                nc.vector.tensor_reduce(
                    st["s"][:, m, :], _re3(expb_sb[m][:]), axis=AX.X, op=ALU.add
                )

        def emit_I(g, st):
            o_sb = spool.tile([128, 4, N], F32, tag="o", name=f"o_{g}")
            nc.vector.tensor_tensor(
                out=o_sb[:], in0=st["s"][:], in1=st["recb"][:], op=ALU.mult
            )
            for m in range(4):
                nc.sync.dma_start(out=outT[g, m, :, :], in_=o_sb[:, m, :])

        # Prologue: all per-group q/ontT/fold/M compute up front (ques/on
        # are tiny and available immediately); the steady-state loop then
        # contains only A/B/CD/F/G/H/I with exactly 64 PSUM allocations per
        # iteration (8-bank aligned) and no serial preamble chains.
        states = {}
        states[0] = pre_dma_small(0)
        for g in range(G):
            if g + 1 < G:
                states[g + 1] = pre_dma_small(g + 1)
            pre_early(g, states[g])
        for g in range(G):
            pre_late(g, states[g])
        pre_dma_big(0, states[0])
        if G > 1:
            pre_dma_big(1, states[1])
        for g in range(G):
            st = states[g]
            stp = states.get(g - 1)
            emit_A(g, st, (0, 1))
            if g + 2 < G:
                pre_dma_big(g + 2, states[g + 2])
            if stp:
                emit_F(g - 1, stp, (0, 1))
            emit_A(g, st, (2, 3))
            if stp:
                emit_F(g - 1, stp, (2, 3))
                emit_G(g - 1, stp)
                emit_H(g - 1, stp, (0, 1))
                emit_H(g - 1, stp, (2, 3))
            emit_B(g, st)
            emit_CD(g, st)
            if stp:
                emit_I(g - 1, stp)
                del states[g - 1]
        st = states[G - 1]
        emit_F(G - 1, st, (0, 1))
        emit_F(G - 1, st, (2, 3))
        emit_G(G - 1, st)
        emit_H(G - 1, st, (0, 1))
        emit_H(G - 1, st, (2, 3))
        emit_I(G - 1, st)

    ctx0.__exit__(None, None, None)
    nsplit = _split_multi_waits(nc)
    if os.environ.get("KERNEL_DEBUG"):
        print(f"split_multi_waits: {nsplit} nops inserted", file=sys.stderr)
    return nc


def _pack_bias(b, dt=np.float32):
    # [H] -> [128, 4]: column j = channels j*128..(j+1)*128
    return np.ascontiguousarray(
        np.asarray(b, np.float32).reshape(4, 128).T.astype(dt)
    )


def _bf(x):
    return np.ascontiguousarray(np.asarray(x, np.float32).astype(ml_dtypes.bfloat16))


def _f8(x):
    return np.ascontiguousarray(
        np.asarray(x, np.float32).astype(ml_dtypes.float8_e4m3fn)
    )


def _pack_planes(x, nplanes):
    """[nplanes*128, F] -> [128, nplanes*F] (plane-major free dim)."""
    x = np.asarray(x)
    K, F = x.shape
    assert K == nplanes * 128
    return np.ascontiguousarray(
        x.reshape(nplanes, 128, F).transpose(1, 0, 2).reshape(128, nplanes * F)
    )


def _smat():
    """[N+1, TOK] node->token selection matrix (+ ones row for the b1 bias)."""
    s = np.zeros((N + 1, TOK), np.float32)
    for n in range(N):
        s[n, n * E : (n + 1) * E] = 1.0
    s[N, :] = 1.0
    return _bf(s)


def prepare_inputs(ques_embed, adj_list, original_nodes,
                   w1_w, w1_b, wq_w, wq_b, we_w, we_b,
                   w2_w, w2_b, wv_w, wv_b, wadj_w, wadj_b):
    """Host-side layout prep. Returns a list of per-core input maps."""
    adjTf = np.asarray(adj_list, np.float32).reshape(BR, TOK, D).transpose(0, 2, 1)
    # adjT with a trailing ones row (badj fold for stage H)
    adjT = np.empty((BR, D + 1, TOK), ml_dtypes.bfloat16)
    adjT[:, :D, :] = adjTf.astype(ml_dtypes.bfloat16)
    adjT[:, D, :] = np.asarray(1.0, ml_dtypes.bfloat16)
    adj8 = np.ascontiguousarray(
        adjTf[:, :256, :].astype(ml_dtypes.float8_e4m3fn)
        .reshape(BR, 2, 128, TOK).transpose(0, 2, 1, 3).reshape(BR, 128, 2 * TOK)
    )
    onT = _bf(
        np.asarray(original_nodes, np.float32).reshape(BR, N, D).transpose(0, 2, 1)
    )
    quesT = _bf(
        np.asarray(ques_embed, np.float32).reshape(BR, 4, 128).transpose(0, 2, 1)
    )

    w1 = np.asarray(w1_w, np.float32)
    w1a = w1[:, :D].T          # [D, H]
    w1b = w1[:, D:].T          # [D, H]
    w2 = np.asarray(w2_w, np.float32)
    wadjT = np.asarray(wadj_w, np.float32).T   # [D, H]
    wadjx = np.concatenate(
        [wadjT, np.asarray(wadj_b, np.float32)[None, :]], axis=0
    )

    w = {
        "w1a32": _bf(SW1 * w1a),
        "w1b8": _pack_planes(_f8(SW1 * w1b[:256]), 2),
        "w1brag32": _bf(SW1 * w1b[256:]),
        "b1row32": _bf(SW1 * np.asarray(w1_b, np.float32).reshape(1, H)),
        "smat": _smat(),
        "wq": _bf(np.asarray(wq_w).T),
        "we8": _pack_planes(_f8(SWE * np.asarray(we_w, np.float32).T), 4),
        "wv8": _pack_planes(_f8(SWV * np.asarray(wv_w, np.float32).T), 4),
        "w2aT": _pack_planes(_bf(w2[:, :D]), 4),
        "w2bT": _pack_planes(_bf(w2[:, D:]), 4),
        "wadjx": _bf(wadjx),
        "bq": _pack_bias(wq_b),
        "be": _pack_bias(we_b),
        "bv": _pack_bias(wv_b),
        "b2c": _pack_bias(w2_b, ml_dtypes.bfloat16),
    }

    in_maps = []
    for c in range(NCORES):
        sl = slice(c * G, (c + 1) * G)
        m = dict(w)
        m["adjT"] = np.ascontiguousarray(adjT[sl])
        m["adj8"] = np.ascontiguousarray(adj8[sl])
        m["onT"] = np.ascontiguousarray(onT[sl])
        m["quesT"] = np.ascontiguousarray(quesT[sl])
        in_maps.append(m)
    return in_maps


def run(in_maps, trace=False, tmpdir=None):
    _install_ntff_hook()
    if not os.environ.get("KERNEL_NO_LDW_DEDUPE"):
        _patch_ldw_dedupe()
    from concourse.bass_utils import run_bass_kernel_spmd

    nc = build_program()
    res = run_bass_kernel_spmd(
        nc,
        in_maps,
        core_ids=list(range(NCORES)),
        trace=trace,
        tmpdir=tmpdir,
    )
    return res


def gather_output(res):
    outT = np.stack([res.results[c]["outT"] for c in range(NCORES)])  # [8,5,4,128,N]
    outT = outT.reshape(BR, 4, 128, N).transpose(0, 3, 1, 2)          # [40,N,4,128]
    return np.ascontiguousarray(outT.reshape(B, R, N, H).astype(np.float32))


def kernel(ques_embed, adj_list, original_nodes,
           w1_w, w1_b, wq_w, wq_b, we_w, we_b,
           w2_w, w2_b, wv_w, wv_b, wadj_w, wadj_b,
           deg=None, batch_size=None, **_unused):
    in_maps = prepare_inputs(
        ques_embed, adj_list, original_nodes,
        w1_w, w1_b, wq_w, wq_b, we_w, we_b,
        w2_w, w2_b, wv_w, wv_b, wadj_w, wadj_b,
    )
    res = run(in_maps, trace=False)
    return gather_output(res)


# revision 29
# speedup vs baseline: 1.1449x; 1.1449x over previous
"""Trainium2 Bass kernel for nn_MessagePassing (gnn_message_passing).

Self-contained: takes full (unsharded) numpy inputs, shards batch*rounds
across 8 NeuronCores, runs a Bass/Tile kernel per core, gathers the full
output.

Math (per (b,r) group, all biases included):
  q      = Wq @ ques + bq                       [H]
  edges  = W1a @ on + W1b @ adj + b1            [H, N*E]  (on broadcast over E)
  a      = softmax_E(We @ (q*edges) + be)
  edges2 = a * edges
  t      = W2a @ adj + W2b @ edges2 + b2
  b      = softmax_E(Wv @ (q*t) + bv)
  out    = sum_E b * (Wadj @ adj + badj)        [H, N]

Design (fp8 DoubleRow + algebraic restructure), HW 325us vs 365us
bf16 baseline, rel err 6.0e-3 (gate 2e-2):
  * Heavy GEMMs (A, B, F) run in fp8 e4m3 with DoubleRow perf mode
    (K=256 per instruction, 2x PE throughput). Stage H (Wadj@adj, the
    only path that touches the output directly) stays bf16.
  * q is folded into the PSUM->SBUF copy scales (per-partition ACT
    scale), so We/Wv are static host-quantized fp8 - no per-group
    weight folds for stages B/F.
  * Stage E (t) is eliminated: t only feeds the b-logits, so
    logits_b = M1 @ adj + M2 @ edges2 + (Wv @ (q*b2)) + bv with
    M1 = Wv diag(q) W2a, M2 = Wv diag(q) W2b folded per group on the
    PE (fp8 DoubleRows over [512,512] - ~7k cycles).
  * badj is folded into stage H's contraction via a ones-row
    (out = recb * sum_E expb*(adj'+badj) works because sum_E b = 1).
  * softmax sums in f32 (DVE reduces run 1x regardless; f32 is free
    accuracy). NOTE: vector.reciprocal with a bf16 input silently
    corrupts results on HW - reciprocal inputs must be f32.
  * all per-group scalar work (q chain, ontT, weight folds, M1/M2,
    bvx) is hoisted into a one-time prologue (ques/on are tiny); the
    steady-state loop is only A/B/CD/F'/G/H/I, interleaved at m-chunk
    granularity, with 64 PSUM allocs per iteration (8-bank aligned).

Layout on device: hidden channels on partitions (4 chunks of 128), tokens
(node*E+e) on the free dim, so softmax over E is a free-dim segment reduce.
fp8 contraction operands are plane-packed: [128, nplanes, F] where plane p
holds contraction rows p*128..(p+1)*128-1.
"""

import os
import sys

for _p in ("/opt/trn_rl_repo", "/root/.axon_site/_ro/trn_rl_repo",
           "/root/.axon_site/_ro/pypackages"):
    if _p not in sys.path and os.path.isdir(_p):
        sys.path.append(_p)

import contextlib
import ctypes
import types

import ml_dtypes
import numpy as np

import concourse.bass as bass
import concourse.tile as tile
from concourse import mybir

BF = mybir.dt.bfloat16
F32 = mybir.dt.float32
F8 = mybir.dt.float8e4
AX = mybir.AxisListType
ALU = mybir.AluOpType
ACTF = mybir.ActivationFunctionType
DR = mybir.MatmulPerfMode.DoubleRow

B, R, N, E, D, H = 4, 10, 80, 20, 300, 512
BR = B * R              # 40 (b,r) groups
NCORES = 8
G = BR // NCORES        # 5 groups per core
TOK = N * E             # 1600 tokens per group
NT = 4                  # token tiles per group
T = TOK // NT           # 400 tokens per tile
DRAG = D - 256          # 44 ragged contraction rows of the D=300 dim
KX = N + 1 + DRAG       # 125: [smat | ones | adj-ragged] packed stage-A chunk

KD = [(0, 128), (128, 256), (256, 300)]               # D=300 contraction chunks
KH = [(0, 128), (128, 256), (256, 384), (384, 512)]   # H=512 contraction chunks
KD1 = [(0, 128), (128, 256), (256, 301)]              # D+1 (wadj+badj row)
MS = [(0, 128), (128, 256), (256, 384), (384, 512)]   # output chunks

# ---- scale constants (see emulate.py for the validated algebra) ----
SW1 = 32.0    # W1 stationary scale (w1a32/w1b8/w1brag32/b1row32)
SE8 = 4.0     # edges8 = SE8*q*edges       (A-copy scale = q*SE8/SW1)
SWE = 64.0    # we8 = SWE*We               (B-exp scale = 1/(SWE*SE8))
SE2 = 16.0    # edges2_8 = SE2*a*edges     (recaq = SE2/(SE8*q*suma))
SWV = 64.0    # wv8 = SWV*Wv
CF = 16.0     # w2aq8/w2bq8 = CF*q*W2xT
CM1 = 8192.0  # m1t8 = CM1*M1T             (copy scale = CM1/(CF*SWV) = 8)
CM2 = 512.0   # m2t8 = CM2*M2T (CM2*SE2 == CM1 so the F psum scales match)
RECQ_CLAMP = 1e4

_MAXW = 1  # this walrus build allows a single semaphore wait per instruction


def _split_multi_waits(nc):
    """Walrus here rejects instructions with >1 sem wait; hoist extra waits
    onto same-engine NoOps inserted just before the instruction."""
    ctr = 0
    for fn in nc.m.functions:
        for bb in fn.blocks:
            new = []
            for inst in bb.instructions:
                si = inst.sync_info
                if si is not None:
                    waits = list(si.on_wait)
                    if len(waits) > _MAXW:
                        for i in range(0, len(waits) - _MAXW, _MAXW):
                            ctr += 1
                            nop = mybir.InstNoOp(name=f"wsplit-{ctr}")
                            nop.engine = inst.engine
                            nop.sync_info = mybir.SyncInfo(
                                on_wait=waits[i : i + _MAXW], on_update=[]
                            )
                            new.append(nop)
                        si.on_wait = waits[len(waits) - _MAXW :]
                new.append(inst)
            bb.instructions = new
    return ctr


def _patch_ldw_dedupe():
    """The bass pipeline splits every matmul into Ldweights + Matmult.
    Consecutive matmuls that share the stationary operand then reload the
    same weights. Drop the redundant Ldweights at the BIR-JSON level
    (walrus's own --enable-ldw-opt rejects explicit Ldweights)."""
    import orjson

    import concourse.bass2jax as b2j
    import concourse.bass_utils as bu

    if getattr(bu, "_ldw_dedupe_patched", False):
        return
    orig = bu.compile_bir_kernel

    def _dedupe(bir_json):
        d = orjson.loads(bir_json)
        removed = 0
        nopctr = 0
        for fn in d.get("functions", []):
            stack = list(fn.get("blocks", []))
            while stack:
                blk = stack.pop()
                stack.extend(blk.get("blocks", []))
                insts = blk.get("instructions", [])
                out = []
                last_key = None
                for i in insts:
                    op = i.get("opcode")
                    if op == "Ldweights":
                        key = orjson.dumps(
                            [
                                i.get("ins"),
                                i.get("perf_mode"),
                                i.get("tile_position"),
                                i.get("tile_size"),
                                i.get("is_transpose"),
                            ]
                        )
                        si = i.get("sync_info") or {}
                        if key == last_key and not si.get("on_update"):
                            w = si.get("on_wait") or []
                            if w:
                                nopctr += 1
                                out.append(
                                    {
                                        "name": f"ldwkeep-{nopctr}",
                                        "opcode": "NoOp",
                                        "engine": i.get("engine", "PE"),
                                        "ins": [],
                                        "outs": [],
                                        "sync_info": {
                                            "on_wait": w,
                                            "on_update": [],
                                        },
                                    }
                                )
                            removed += 1
                            continue
                        last_key = key
                    elif op == "Matmult":
                        if i.get("is_transpose") or i.get("ldweights"):
                            last_key = None
                    out.append(i)
                blk["instructions"] = out
        if os.environ.get("KERNEL_DEBUG"):
            print(f"ldw dedupe: removed {removed}", file=sys.stderr)
        return orjson.dumps(d)

    def compile_bir_kernel(bir_json, tmpdir, neff_name="file.neff"):
        try:
            bir_json = _dedupe(bir_json)
        except Exception as e:  # pragma: no cover - safety net
            print(f"ldw dedupe skipped: {e}", file=sys.stderr)
        return orig(bir_json, tmpdir, neff_name=neff_name)

    bu.compile_bir_kernel = compile_bir_kernel
    b2j.compile_bir_kernel = compile_bir_kernel
    bu._ldw_dedupe_patched = True


def _install_ntff_hook():
    """Provide antenv.axon_hooks (missing in this image) so that
    run_bass_kernel_spmd(trace=True) can profile via libaxon_pjrt."""
    if "antenv.axon_hooks" in sys.modules:
        return

    def _mk(so_path):
        try:
            lib = ctypes.CDLL(so_path)
        except OSError:
            return None
        if not hasattr(lib, "axon_start_nrt_profile"):
            return None
        lib.axon_start_nrt_profile.argtypes = [
            ctypes.POINTER(ctypes.c_int64),
            ctypes.c_size_t,
        ]
        lib.axon_start_nrt_profile.restype = ctypes.c_int64
        lib.axon_stop_nrt_profile.argtypes = [ctypes.c_char_p]
        lib.axon_stop_nrt_profile.restype = ctypes.c_int64

        @contextlib.contextmanager
        def _hook(output_dir, device_ids):
            import jax

            jax.devices()
            if device_ids:
                ids = (ctypes.c_int64 * len(device_ids))(*device_ids)
                rc = lib.axon_start_nrt_profile(ids, len(device_ids))
            else:
                rc = lib.axon_start_nrt_profile(None, 0)
            if rc != 0:
                raise RuntimeError(f"axon_start_nrt_profile rc={rc}")
            try:
                yield
            finally:
                n = lib.axon_stop_nrt_profile(str(output_dir).encode())
                print(f"ntff profile: {n} file(s) -> {output_dir}", file=sys.stderr)

        return _hook

    hook = _mk("/opt/axon/libaxon_pjrt.so")
    mod = types.ModuleType("antenv.axon_hooks")
    mod.get_axon_ntff_profile_hook = lambda: hook
    try:
        import antenv

        antenv.axon_hooks = mod
    except ImportError:
        pass
    sys.modules["antenv.axon_hooks"] = mod

    import concourse.bass_utils as bass_utils

    bass_utils.upload_artifacts = lambda tmpdir: f"local://{tmpdir}"


def _re3(ap):
    """[128, n*E] -> [128, n, E] view."""
    return ap.rearrange("p (n e) -> p n e", e=E)


def build_program():
    nc = bass.Bass()

    # --- per-group data ---
    adjT = nc.declare_dram_parameter("adjT", [G, D + 1, TOK], BF, isOutput=False)
    adj8_d = nc.declare_dram_parameter("adj8", [G, 128, 2 * TOK], F8, isOutput=False)
    onT = nc.declare_dram_parameter("onT", [G, D, N], BF, isOutput=False)
    quesT = nc.declare_dram_parameter("quesT", [G, 128, 4], BF, isOutput=False)
    # --- static weights ---
    w1a32_d = nc.declare_dram_parameter("w1a32", [D, H], BF, isOutput=False)
    w1b8_d = nc.declare_dram_parameter("w1b8", [128, 2 * H], F8, isOutput=False)
    w1brag32_d = nc.declare_dram_parameter("w1brag32", [DRAG, H], BF, isOutput=False)
    b1row32_d = nc.declare_dram_parameter("b1row32", [1, H], BF, isOutput=False)
    smat_d = nc.declare_dram_parameter("smat", [N + 1, TOK], BF, isOutput=False)
    wq_d = nc.declare_dram_parameter("wq", [H, H], BF, isOutput=False)
    we8_d = nc.declare_dram_parameter("we8", [128, 4 * H], F8, isOutput=False)
    wv8_d = nc.declare_dram_parameter("wv8", [128, 4 * H], F8, isOutput=False)
    w2aT_d = nc.declare_dram_parameter("w2aT", [128, 4 * D], BF, isOutput=False)
    w2bT_d = nc.declare_dram_parameter("w2bT", [128, 4 * H], BF, isOutput=False)
    wadjx_d = nc.declare_dram_parameter("wadjx", [D + 1, H], BF, isOutput=False)
    # biases packed [128, 4] (column j = channels j*128..j*128+127)
    bq_d = nc.declare_dram_parameter("bq", [128, 4], F32, isOutput=False)
    be_d = nc.declare_dram_parameter("be", [128, 4], F32, isOutput=False)
    bv_d = nc.declare_dram_parameter("bv", [128, 4], F32, isOutput=False)
    b2c_d = nc.declare_dram_parameter("b2c", [128, 4], BF, isOutput=False)

    outT = nc.declare_dram_parameter("outT", [G, 4, 128, N], F32, isOutput=True)

    def tsl(t):
        return slice(t * T, (t + 1) * T)

    ctx0 = nc.allow_low_precision("softmax sums kept in bf16 deliberately")
    ctx0.__enter__()
    with tile.TileContext(nc) as tc, contextlib.ExitStack() as ctx:
        wpool = ctx.enter_context(tc.tile_pool(name="weights", bufs=1))
        gpool = ctx.enter_context(tc.tile_pool(name="group", bufs=2))
        gpool3 = ctx.enter_context(tc.tile_pool(name="group3", bufs=3))
        spool = ctx.enter_context(tc.tile_pool(name="small", bufs=2))
        spool3 = ctx.enter_context(tc.tile_pool(name="small3", bufs=3))
        ppool = ctx.enter_context(tc.tile_pool(name="pergroup", bufs=G))
        pspool = ctx.enter_context(tc.tile_pool(name="ps", bufs=8, space="PSUM"))

        # PE warmup: keep the HAM clock-gate at 8/8 through the startup
        # DMA wait so the first real matmuls run at 2.4 GHz.
        wu_sb = wpool.tile([128, 512], BF, tag="wu", name="wu")
        nc.vector.memset(wu_sb[:], 0.0)
        wu_ps = pspool.tile([128, T], F32, tag="ps", name="wups")
        for i in range(85):
            nc.tensor.matmul(
                wu_ps[:], wu_sb[:, :128], wu_sb[:, :T], start=True, stop=True
            )

        def load_w(dram, shape, dt_, name):
            t_ = wpool.tile(shape, dt_, tag=name, name=name)
            nc.scalar.dma_start(out=t_[:], in_=dram[:, :])
            return t_

        def load_w_chunks(dram, chunks, name):
            tiles = []
            for ki, (k0, k1) in enumerate(chunks):
                t_ = wpool.tile(
                    [k1 - k0, H], BF, tag=f"{name}{ki}", name=f"{name}{ki}"
                )
                nc.scalar.dma_start(out=t_[:], in_=dram[k0:k1, :])
                tiles.append(t_)
            return tiles

        w1a32_sb = load_w_chunks(w1a32_d, KD, "w1a32")
        wq_sb = load_w_chunks(wq_d, KH, "wq")
        wadjx_sb = load_w_chunks(wadjx_d, KD1, "wadjx")
        w1b8_sb = load_w(w1b8_d, [128, 2, H], F8, "w1b8")
        we8_sb = load_w(we8_d, [128, 4, H], F8, "we8")
        wv8_sb = load_w(wv8_d, [128, 4, H], F8, "wv8")
        w2aT_sb = load_w(w2aT_d, [128, 4, D], BF, "w2aT")
        w2bT_sb = load_w(w2bT_d, [128, 4, H], BF, "w2bT")
        bq_sb = load_w(bq_d, [128, 4], F32, "bq")
        be_sb = load_w(be_d, [128, 4], F32, "be")
        bv_sb = load_w(bv_d, [128, 4], F32, "bv")
        b2c_sb = load_w(b2c_d, [128, 4], BF, "b2c")

        def pre_dma_small(g):
            """Prologue DMAs: ques/on + the static rows of w1x."""
            st = {}
            ques_sb = spool.tile([128, 4], BF, tag="ques", name=f"ques_{g}")
            nc.sync.dma_start(out=ques_sb[:], in_=quesT[g, :, :])
            on_sb = []
            for ki, (k0, k1) in enumerate(KD):
                t_ = spool.tile(
                    [k1 - k0, N], BF, tag=f"on{ki}", name=f"on{ki}_{g}"
                )
                nc.sync.dma_start(out=t_[:], in_=onT[g, k0:k1, :])
                on_sb.append(t_)
            w1x_sb = ppool.tile([KX, H], BF, tag="w1x", name=f"w1x_{g}")
            nc.sync.dma_start(out=w1x_sb[N : N + 1, :], in_=b1row32_d[:, :])
            nc.sync.dma_start(out=w1x_sb[N + 1 :, :], in_=w1brag32_d[:, :])
            st["ques"] = ques_sb
            st["on"] = on_sb
            st["w1x"] = w1x_sb
            return st

        def pre_dma_big(g, st):
            """Per-iteration DMAs: adjacency tensors."""
            adjx_sb = gpool3.tile([KX, TOK], BF, tag="adjx", name=f"adjx_{g}")
            nc.sync.dma_start(out=adjx_sb[: N + 1, :], in_=smat_d[:, :])
            nc.sync.dma_start(out=adjx_sb[N + 1 :, :], in_=adjT[g, 256:D, :])
            adj8_sb = gpool3.tile([128, 2, TOK], F8, tag="adj8", name=f"adj8_{g}")
            nc.sync.dma_start(out=adj8_sb[:], in_=adj8_d[g, :, :])
            adjc = []
            for ki, (k0, k1) in enumerate(KD1):
                t_ = gpool3.tile(
                    [k1 - k0, TOK], BF, tag=f"adj{ki}", name=f"adj{ki}_{g}"
                )
                nc.sync.dma_start(out=t_[:], in_=adjT[g, k0:k1, :])
                adjc.append(t_)
            st["adjx"] = adjx_sb
            st["adj8"] = adj8_sb
            st["adjc"] = adjc

        def pre_early(g, st):
            """q chain + ontT + ACT weight folds (no M matmuls yet)."""
            ques_sb = st["ques"]
            on_sb = st["on"]
            w1x_sb = st["w1x"]

            # q = Wq @ ques + bq  (f32, kept for copy scales only)
            q_ps = pspool.tile([128, 4], F32, tag="ps", name=f"qps_{g}")
            for m, (m0, m1) in enumerate(MS):
                for k in range(4):
                    nc.tensor.matmul(
                        q_ps[:, m : m + 1],
                        wq_sb[k][:, m0:m1],
                        ques_sb[:, k : k + 1],
                        start=(k == 0),
                        stop=(k == 3),
                    )
            q_sb = spool.tile([128, 4], F32, tag="q", name=f"q_{g}")
            for m in range(4):
                nc.scalar.activation(
                    out=q_sb[:, m : m + 1],
                    in_=q_ps[:, m : m + 1],
                    func=ACTF.Identity,
                    bias=bq_sb[:, m : m + 1],
                )
            qA_sb = ppool.tile([128, 4], F32, tag="qA", name=f"qA_{g}")
            nc.scalar.mul(qA_sb[:], q_sb[:], SE8 / SW1)
            qC_sb = spool.tile([128, 4], F32, tag="qC", name=f"qC_{g}")
            nc.scalar.mul(qC_sb[:], q_sb[:], CF)
            rq_sb = spool.tile([128, 4], F32, tag="rq", name=f"rq_{g}")
            nc.vector.reciprocal(rq_sb[:], q_sb[:])
            recq_sb = spool.tile([128, 4], F32, tag="recq", name=f"recq_{g}")
            nc.vector.tensor_scalar(
                out=recq_sb[:],
                in0=rq_sb[:],
                scalar1=RECQ_CLAMP,
                scalar2=-RECQ_CLAMP,
                op0=ALU.min,
                op1=ALU.max,
            )
            # recqf = recq*(SE2/SE8): folded scale for the softmax-a chain
            recqf_sb = ppool.tile([128, 4], F32, tag="recqf", name=f"recqf_{g}")
            nc.scalar.mul(recqf_sb[:], recq_sb[:], SE2 / SE8)
            st["qA"] = qA_sb
            st["qC"] = qC_sb
            st["recqf"] = recqf_sb

            # transposed on-term: ontT[n, c] = SW1 * sum_f on[f, n] W1a[f, c]
            ontT_ps = pspool.tile([N, H], F32, tag="ps", name=f"ontTps_{g}")
            for ki in range(3):
                nc.tensor.matmul(
                    ontT_ps[:],
                    on_sb[ki][:],
                    w1a32_sb[ki][:],
                    start=(ki == 0),
                    stop=(ki == 2),
                )
            nc.scalar.copy(out=w1x_sb[:N, :], in_=ontT_ps[:])

            # folds: w2aq8 = CF*q (.) w2aT ; w2bq8 = CF*q (.) w2bT   (ACT)
            w2aq8 = ppool.tile([128, 4, H], F8, tag="w2aq8", name=f"w2aq8_{g}")
            w2bq8 = spool3.tile([128, 4, H], F8, tag="w2bq8", name=f"w2bq8_{g}")
            for k in range(4):
                nc.scalar.activation(
                    out=w2aq8[:, k, :D], in_=w2aT_sb[:, k, :],
                    func=ACTF.Copy, scale=qC_sb[:, k : k + 1],
                )
                nc.scalar.activation(
                    out=w2bq8[:, k, :], in_=w2bT_sb[:, k, :],
                    func=ACTF.Copy, scale=qC_sb[:, k : k + 1],
                )
            st["w2aq8"] = w2aq8
            st["w2bq8"] = w2bq8

        def pre_late(g, st):
            """M1/M2 fold matmuls + fp8 copies + b2 bias fold."""
            qC_sb = st["qC"]
            w2aq8 = st["w2aq8"]
            w2bq8 = st["w2bq8"]

            m1t8 = ppool.tile([128, 2, H], F8, tag="m1t8", name=f"m1t8_{g}")
            m1trag = ppool.tile([DRAG, H], BF, tag="m1trag", name=f"m1trag_{g}")
            for dc, (d0, d1) in enumerate(KD):
                psM = pspool.tile([d1 - d0, H], F32, tag="ps", name=f"m1ps_{g}_{dc}")
                for i in range(2):
                    nc.tensor.matmul(
                        psM[:],
                        w2aq8[:, 2 * i : 2 * i + 2, d0:d1],
                        wv8_sb[:, 2 * i : 2 * i + 2, :],
                        start=(i == 0),
                        stop=(i == 1),
                        perf_mode=DR,
                    )
                if dc < 2:
                    nc.vector.tensor_scalar_mul(
                        m1t8[:, dc, :], psM[:], CM1 / (CF * SWV)
                    )
                else:
                    nc.vector.tensor_scalar_mul(
                        m1trag[:, :], psM[:], CM1 / (CF * SWV)
                    )
            m2t8a = ppool.tile([128, 2, H], F8, tag="m2t8a", name=f"m2t8a_{g}")
            m2t8b = ppool.tile([128, 2, H], F8, tag="m2t8b", name=f"m2t8b_{g}")
            for kc, (k0, k1) in enumerate(KH):
                psM = pspool.tile([128, H], F32, tag="ps", name=f"m2ps_{g}_{kc}")
                for i in range(2):
                    nc.tensor.matmul(
                        psM[:],
                        w2bq8[:, 2 * i : 2 * i + 2, k0:k1],
                        wv8_sb[:, 2 * i : 2 * i + 2, :],
                        start=(i == 0),
                        stop=(i == 1),
                        perf_mode=DR,
                    )
                dst = m2t8a if kc < 2 else m2t8b
                nc.vector.tensor_scalar_mul(
                    dst[:, kc % 2, :], psM[:], CM2 / (CF * SWV)
                )
            st["m1t8"] = m1t8
            st["m1trag"] = m1trag
            st["m2t8a"] = m2t8a
            st["m2t8b"] = m2t8b

            # b2 correction folded into the F-exp bias:
            # bvx = bv + Wv @ (q*b2) = bv + wv8^T @ qb2_8 / (SWV*CF)
            qb2_8 = spool.tile([128, 4], F8, tag="qb2", name=f"qb2_{g}")
            nc.vector.tensor_tensor(
                out=qb2_8[:], in0=qC_sb[:], in1=b2c_sb[:], op=ALU.mult
            )
            qb2_ps = pspool.tile([128, 4], F32, tag="ps", name=f"qb2ps_{g}")
            for m, (m0, m1) in enumerate(MS):
                for k in range(4):
                    nc.tensor.matmul(
                        qb2_ps[:, m : m + 1],
                        wv8_sb[:, k, m0:m1],
                        qb2_8[:, k : k + 1],
                        start=(k == 0),
                        stop=(k == 3),
                    )
            bvx_sb = ppool.tile([128, 4], F32, tag="bvx", name=f"bvx_{g}")
            for m in range(4):
                nc.scalar.activation(
                    out=bvx_sb[:, m : m + 1],
                    in_=qb2_ps[:, m : m + 1],
                    func=ACTF.Identity,
                    scale=1.0 / (SWV * CF),
                    bias=bv_sb[:, m : m + 1],
                )
            st["bvx"] = bvx_sb

        def emit_A(g, st, ms):
            """Stage A (edges psum + fp8 copy) for m-chunks in ms."""
            adj8_sb = st["adj8"]
            w1x_sb = st["w1x"]
            adjx_sb = st["adjx"]
            qA_sb = st["qA"]
            if "edges8" not in st:
                st["edges8"] = gpool.tile(
                    [128, 4, TOK], F8, tag="edges8", name=f"edges8_{g}"
                )
                st["expa"] = [
                    gpool.tile([128, TOK], BF, tag=f"expa{m}", name=f"expa{m}_{g}")
                    for m in range(4)
                ]
                st["e2"] = gpool.tile([128, 4, TOK], F8, tag="e2", name=f"e2_{g}")
            edges8 = st["edges8"]

            for m in ms:
                m0, m1 = MS[m]
                eps = [
                    pspool.tile([128, T], F32, tag="ps", name=f"eps_{g}_{m}_{t}")
                    for t in range(NT)
                ]
                for t in range(NT):
                    nc.tensor.matmul(
                        eps[t][:],
                        w1b8_sb[:, :, m0:m1],
                        adj8_sb[:, :, tsl(t)],
                        start=True,
                        stop=False,
                        perf_mode=DR,
                    )
                for t in range(NT):
                    nc.tensor.matmul(
                        eps[t][:],
                        w1x_sb[:, m0:m1],
                        adjx_sb[:, tsl(t)],
                        start=False,
                        stop=True,
                    )
                # A-copy (ACT): edges8 = psum * (q*SE8/SW1), fp8 out
                for t in range(NT):
                    nc.scalar.activation(
                        out=edges8[:, m, tsl(t)],
                        in_=eps[t][:],
                        func=ACTF.Copy,
                        scale=qA_sb[:, m : m + 1],
                    )

        def emit_B(g, st):
            """Stage B: expa = exp(psum/(SWE*SE8) + be)."""
            edges8 = st["edges8"]
            expa_sb = st["expa"]
            for m, (m0, m1) in enumerate(MS):
                lps = [
                    pspool.tile([128, T], F32, tag="ps", name=f"lps_{g}_{m}_{t}")
                    for t in range(NT)
                ]
                for i in range(2):
                    for t in range(NT):
                        nc.tensor.matmul(
                            lps[t][:],
                            we8_sb[:, 2 * i : 2 * i + 2, m0:m1],
                            edges8[:, 2 * i : 2 * i + 2, tsl(t)],
                            start=(i == 0),
                            stop=(i == 1),
                            perf_mode=DR,
                        )
                for t in range(NT):
                    nc.scalar.activation(
                        out=expa_sb[m][:, tsl(t)],
                        in_=lps[t][:],
                        func=ACTF.Exp,
                        scale=1.0 / (SWE * SE8),
                        bias=be_sb[:, m : m + 1],
                    )

        def emit_CD(g, st):
            """softmax-a chain + edges2 per m-chunk."""
            edges8 = st["edges8"]
            expa_sb = st["expa"]
            e2_8 = st["e2"]
            recqf_sb = st["recqf"]
            for m in range(4):
                suma = spool.tile([128, N], F32, tag=f"suma{m}", name=f"suma{m}_{g}")
                nc.vector.tensor_reduce(
                    suma[:], _re3(expa_sb[m][:]), axis=AX.X, op=ALU.add
                )
                rs = spool.tile([128, N], F32, tag=f"rs{m}", name=f"rs{m}_{g}")
                nc.vector.reciprocal(rs[:], suma[:])
                recaq = spool.tile(
                    [128, N], F32, tag=f"recaq{m}", name=f"recaq{m}_{g}"
                )
                nc.vector.tensor_scalar_mul(recaq[:], rs[:], recqf_sb[:, m : m + 1])
                # aN = expa * recaq (broadcast over E) in place   (Pool)
                nc.gpsimd.tensor_tensor(
                    out=_re3(expa_sb[m][:]),
                    in0=_re3(expa_sb[m][:]),
                    in1=recaq[:, :, None].broadcast_to((128, N, E)),
                    op=ALU.mult,
                )
                # edges2_8 = aN * edges8 (Pool, fp8 out)
                nc.gpsimd.tensor_tensor(
                    out=e2_8[:, m, :],
                    in0=expa_sb[m][:],
                    in1=edges8[:, m, :],
                    op=ALU.mult,
                )

        def emit_F(g, st, ms):
            """Stage F' (logits_b via M1/M2) + expb for m-chunks in ms."""
            adjc = st["adjc"]
            adj8_sb = st["adj8"]
            e2_8 = st["e2"]
            m1t8 = st["m1t8"]
            m1trag = st["m1trag"]
            m2t8a = st["m2t8a"]
            m2t8b = st["m2t8b"]
            bvx_sb = st["bvx"]
            if "expb" not in st:
                st["expb"] = [
                    gpool.tile([128, TOK], BF, tag=f"expb{m}", name=f"expb{m}_{g}")
                    for m in range(4)
                ]
            expb_sb = st["expb"]

            for m in ms:
                m0, m1 = MS[m]
                bps = [
                    pspool.tile([128, T], F32, tag="ps", name=f"bps_{g}_{m}_{t}")
                    for t in range(NT)
                ]
                for t in range(NT):
                    nc.tensor.matmul(
                        bps[t][:],
                        m1t8[:, :, m0:m1],
                        adj8_sb[:, :, tsl(t)],
                        start=True,
                        stop=False,
                        perf_mode=DR,
                    )
                for t in range(NT):
                    nc.tensor.matmul(
                        bps[t][:],
                        m1trag[:, m0:m1],
                        adjc[2][:DRAG, tsl(t)],
                        start=False,
                        stop=False,
                    )
                for t in range(NT):
                    nc.tensor.matmul(
                        bps[t][:],
                        m2t8a[:, :, m0:m1],
                        e2_8[:, 0:2, tsl(t)],
                        start=False,
                        stop=False,
                        perf_mode=DR,
                    )
                for t in range(NT):
                    nc.tensor.matmul(
                        bps[t][:],
                        m2t8b[:, :, m0:m1],
                        e2_8[:, 2:4, tsl(t)],
                        start=False,
                        stop=True,
                        perf_mode=DR,
                    )
                for t in range(NT):
                    nc.scalar.activation(
                        out=expb_sb[m][:, tsl(t)],
                        in_=bps[t][:],
                        func=ACTF.Exp,
                        scale=1.0 / CM1,
                        bias=bvx_sb[:, m : m + 1],
                    )

        def emit_G(g, st):
            expb_sb = st["expb"]
            sumb = spool.tile([128, 4, N], F32, tag="sumb", name=f"sumb_{g}")
            for m in range(4):
                nc.vector.tensor_reduce(
                    sumb[:, m, :], _re3(expb_sb[m][:]), axis=AX.X, op=ALU.add
                )
            recb = spool.tile([128, 4, N], F32, tag="recb", name=f"recb_{g}")
            nc.vector.reciprocal(recb[:], sumb[:])
            st["recb"] = recb
            st["s"] = spool.tile([128, 4, N], F32, tag="s", name=f"s_{g}")

        def emit_H(g, st, ms):
            """Stage H (bf16, badj via ones-row) for m-chunks in ms."""
            adjc = st["adjc"]
            expb_sb = st["expb"]
            for m in ms:
                m0, m1 = MS[m]
                aps = [
                    pspool.tile([128, T], F32, tag="ps", name=f"aps_{g}_{m}_{t}")
                    for t in range(NT)
                ]
                for ki in range(3):
                    for t in range(NT):
                        nc.tensor.matmul(
                            aps[t][:],
                            wadjx_sb[ki][:, m0:m1],
                            adjc[ki][:, tsl(t)],
                            start=(ki == 0),
                            stop=(ki == 2),
                        )
                # pre = psum * expb  (DVE - GPSIMD cannot access PSUM)
                for t in range(NT):
                    nc.vector.tensor_tensor(
                        out=expb_sb[m][:, tsl(t)],
                        in0=aps[t][:],
                        in1=expb_sb[m][:, tsl(t)],
                        op=ALU.mult,
                    )
                nc.vector.tensor_reduce(
                    st["s"][:, m, :], _re3(expb_sb[m][:]), axis=AX.X, op=ALU.add
                )

        def emit_I(g, st):
            o_sb = spool.tile([128, 4, N], F32, tag="o", name=f"o_{g}")
            nc.vector.tensor_tensor(
                out=o_sb[:], in0=st["s"][:], in1=st["recb"][:], op=ALU.mult
            )
            for m in range(4):
                nc.sync.dma_start(out=outT[g, m, :, :], in_=o_sb[:, m, :])

        # Prologue: all per-group q/ontT/fold/M compute up front (ques/on
        # are tiny and available immediately); the steady-state loop then
        # contains only A/B/CD/F/G/H/I with exactly 64 PSUM allocations per
        # iteration (8-bank aligned) and no serial preamble chains.
        states = {}
        states[0] = pre_dma_small(0)
        for g in range(G):
            if g + 1 < G:
                states[g + 1] = pre_dma_small(g + 1)
            pre_early(g, states[g])
        for g in range(G):
            pre_late(g, states[g])
        pre_dma_big(0, states[0])
        if G > 1:
            pre_dma_big(1, states[1])
        for g in range(G):
            st = states[g]
            stp = states.get(g - 1)
            emit_A(g, st, (0, 1))
            if g + 2 < G:
                pre_dma_big(g + 2, states[g + 2])
            if stp:
                emit_F(g - 1, stp, (0, 1))
            emit_A(g, st, (2, 3))
            if stp:
                emit_F(g - 1, stp, (2, 3))
                emit_G(g - 1, stp)
            emit_B(g, st)
            if stp:
                emit_H(g - 1, stp, (0, 1))
            emit_CD(g, st)
            if stp:
                emit_H(g - 1, stp, (2, 3))
                emit_I(g - 1, stp)
                del states[g - 1]
        st = states[G - 1]
        emit_F(G - 1, st, (0, 1))
        emit_F(G - 1, st, (2, 3))
        emit_G(G - 1, st)
        emit_H(G - 1, st, (0, 1))
        emit_H(G - 1, st, (2, 3))
        emit_I(G - 1, st)

    ctx0.__exit__(None, None, None)
    nsplit = _split_multi_waits(nc)
    if os.environ.get("KERNEL_DEBUG"):
        print(f"split_multi_waits: {nsplit} nops inserted", file=sys.stderr)
    return nc


def _pack_bias(b, dt=np.float32):
    # [H] -> [128, 4]: column j = channels j*128..(j+1)*128
    return np.ascontiguousarray(
        np.asarray(b, np.float32).reshape(4, 128).T.astype(dt)
    )


def _bf(x):
    return np.ascontiguousarray(np.asarray(x, np.float32).astype(ml_dtypes.bfloat16))


def _f8(x):
    return np.ascontiguousarray(
        np.asarray(x, np.float32).astype(ml_dtypes.float8_e4m3fn)
    )


def _pack_planes(x, nplanes):
    """[nplanes*128, F] -> [128, nplanes*F] (plane-major free dim)."""
    x = np.asarray(x)
    K, F = x.shape
    assert K == nplanes * 128
    return np.ascontiguousarray(
        x.reshape(nplanes, 128, F).transpose(1, 0, 2).reshape(128, nplanes * F)
    )


def _smat():
    """[N+1, TOK] node->token selection matrix (+ ones row for the b1 bias)."""
    s = np.zeros((N + 1, TOK), np.float32)
    for n in range(N):
        s[n, n * E : (n + 1) * E] = 1.0
    s[N, :] = 1.0
    return _bf(s)


def prepare_inputs(ques_embed, adj_list, original_nodes,
                   w1_w, w1_b, wq_w, wq_b, we_w, we_b,
                   w2_w, w2_b, wv_w, wv_b, wadj_w, wadj_b):
    """Host-side layout prep. Returns a list of per-core input maps."""
    adjTf = np.asarray(adj_list, np.float32).reshape(BR, TOK, D).transpose(0, 2, 1)
    # adjT with a trailing ones row (badj fold for stage H)
    adjT = np.empty((BR, D + 1, TOK), ml_dtypes.bfloat16)
    adjT[:, :D, :] = adjTf.astype(ml_dtypes.bfloat16)
    adjT[:, D, :] = np.asarray(1.0, ml_dtypes.bfloat16)
    adj8 = np.ascontiguousarray(
        adjTf[:, :256, :].astype(ml_dtypes.float8_e4m3fn)
        .reshape(BR, 2, 128, TOK).transpose(0, 2, 1, 3).reshape(BR, 128, 2 * TOK)
    )
    onT = _bf(
        np.asarray(original_nodes, np.float32).reshape(BR, N, D).transpose(0, 2, 1)
    )
    quesT = _bf(
        np.asarray(ques_embed, np.float32).reshape(BR, 4, 128).transpose(0, 2, 1)
    )

    w1 = np.asarray(w1_w, np.float32)
    w1a = w1[:, :D].T          # [D, H]
    w1b = w1[:, D:].T          # [D, H]
    w2 = np.asarray(w2_w, np.float32)
    wadjT = np.asarray(wadj_w, np.float32).T   # [D, H]
    wadjx = np.concatenate(
        [wadjT, np.asarray(wadj_b, np.float32)[None, :]], axis=0
    )

    w = {
        "w1a32": _bf(SW1 * w1a),
        "w1b8": _pack_planes(_f8(SW1 * w1b[:256]), 2),
        "w1brag32": _bf(SW1 * w1b[256:]),
        "b1row32": _bf(SW1 * np.asarray(w1_b, np.float32).reshape(1, H)),
        "smat": _smat(),
        "wq": _bf(np.asarray(wq_w).T),
        "we8": _pack_planes(_f8(SWE * np.asarray(we_w, np.float32).T), 4),
        "wv8": _pack_planes(_f8(SWV * np.asarray(wv_w, np.float32).T), 4),
        "w2aT": _pack_planes(_bf(w2[:, :D]), 4),
        "w2bT": _pack_planes(_bf(w2[:, D:]), 4),
        "wadjx": _bf(wadjx),
        "bq": _pack_bias(wq_b),
        "be": _pack_bias(we_b),
        "bv": _pack_bias(wv_b),
        "b2c": _pack_bias(w2_b, ml_dtypes.bfloat16),
    }

    in_maps = []
    for c in range(NCORES):
        sl = slice(c * G, (c + 1) * G)
        m = dict(w)
        m["adjT"] = np.ascontiguousarray(adjT[sl])
        m["adj8"] = np.ascontiguousarray(adj8[sl])
        m["onT"] = np.ascontiguousarray(onT[sl])
        m["quesT"] = np.ascontiguousarray(quesT[sl])
        in_maps.append(m)
    return in_maps


def run(in_maps, trace=False, tmpdir=None):
    _install_ntff_hook()
    if not os.environ.get("KERNEL_NO_LDW_DEDUPE"):
        _patch_ldw_dedupe()
    from concourse.bass_utils import run_bass_kernel_spmd

    nc = build_program()
    res = run_bass_kernel_spmd(
        nc,
        in_maps,
        core_ids=list(range(NCORES)),
        trace=trace,
        tmpdir=tmpdir,
    )
    return res


def gather_output(res):
    outT = np.stack([res.results[c]["outT"] for c in range(NCORES)])  # [8,5,4,128,N]
    outT = outT.reshape(BR, 4, 128, N).transpose(0, 3, 1, 2)          # [40,N,4,128]
    return np.ascontiguousarray(outT.reshape(B, R, N, H).astype(np.float32))


def kernel(ques_embed, adj_list, original_nodes,
           w1_w, w1_b, wq_w, wq_b, we_w, we_b,
           w2_w, w2_b, wv_w, wv_b, wadj_w, wadj_b,
           deg=None, batch_size=None, **_unused):
    in_maps = prepare_inputs(
        ques_embed, adj_list, original_nodes,
        w1_w, w1_b, wq_w, wq_b, we_w, we_b,
        w2_w, w2_b, wv_w, wv_b, wadj_w, wadj_b,
    )
    res = run(in_maps, trace=False)
    return gather_output(res)


# revision 30
# speedup vs baseline: 1.2164x; 1.0625x over previous
"""Trainium2 Bass kernel for nn_MessagePassing (gnn_message_passing).

Self-contained: takes full (unsharded) numpy inputs, shards batch*rounds
across 8 NeuronCores, runs a Bass/Tile kernel per core, gathers the full
output.

Math (per (b,r) group, all biases included):
  q      = Wq @ ques + bq                       [H]
  edges  = W1a @ on + W1b @ adj + b1            [H, N*E]  (on broadcast over E)
  a      = softmax_E(We @ (q*edges) + be)
  edges2 = a * edges
  t      = W2a @ adj + W2b @ edges2 + b2
  b      = softmax_E(Wv @ (q*t) + bv)
  out    = sum_E b * (Wadj @ adj + badj)        [H, N]

Design (fp8 DoubleRow + algebraic restructure), HW 325us vs 365us
bf16 baseline, rel err 6.0e-3 (gate 2e-2):
  * Heavy GEMMs (A, B, F) run in fp8 e4m3 with DoubleRow perf mode
    (K=256 per instruction, 2x PE throughput). Stage H (Wadj@adj, the
    only path that touches the output directly) stays bf16.
  * q is folded into the PSUM->SBUF copy scales (per-partition ACT
    scale), so We/Wv are static host-quantized fp8 - no per-group
    weight folds for stages B/F.
  * Stage E (t) is eliminated: t only feeds the b-logits, so
    logits_b = M1 @ adj + M2 @ edges2 + (Wv @ (q*b2)) + bv with
    M1 = Wv diag(q) W2a, M2 = Wv diag(q) W2b folded per group on the
    PE (fp8 DoubleRows over [512,512] - ~7k cycles).
  * badj is folded into stage H's contraction via a ones-row
    (out = recb * sum_E expb*(adj'+badj) works because sum_E b = 1).
  * softmax sums in f32 (DVE reduces run 1x regardless; f32 is free
    accuracy). NOTE: vector.reciprocal with a bf16 input silently
    corrupts results on HW - reciprocal inputs must be f32.
  * all per-group scalar work (q chain, ontT, weight folds, M1/M2,
    bvx) is hoisted into a one-time prologue (ques/on are tiny); the
    steady-state loop is only A/B/CD/F'/G/H/I, interleaved at m-chunk
    granularity, with 64 PSUM allocs per iteration (8-bank aligned).

Layout on device: hidden channels on partitions (4 chunks of 128), tokens
(node*E+e) on the free dim, so softmax over E is a free-dim segment reduce.
fp8 contraction operands are plane-packed: [128, nplanes, F] where plane p
holds contraction rows p*128..(p+1)*128-1.
"""

import os
import sys

for _p in ("/opt/trn_rl_repo", "/root/.axon_site/_ro/trn_rl_repo",
           "/root/.axon_site/_ro/pypackages"):
    if _p not in sys.path and os.path.isdir(_p):
        sys.path.append(_p)

import contextlib
import ctypes
import types

import ml_dtypes
import numpy as np

import concourse.bass as bass
import concourse.tile as tile
from concourse import mybir

BF = mybir.dt.bfloat16
F32 = mybir.dt.float32
F8 = mybir.dt.float8e4
AX = mybir.AxisListType
ALU = mybir.AluOpType
ACTF = mybir.ActivationFunctionType
DR = mybir.MatmulPerfMode.DoubleRow

B, R, N, E, D, H = 4, 10, 80, 20, 300, 512
BR = B * R              # 40 (b,r) groups
NCORES = 8
G = BR // NCORES        # 5 groups per core
TOK = N * E             # 1600 tokens per group
NT = 4                  # token tiles per group
T = TOK // NT           # 400 tokens per tile
DRAG = D - 256          # 44 ragged contraction rows of the D=300 dim
KX = N + 1 + DRAG       # 125: [smat | ones | adj-ragged] packed stage-A chunk

KD = [(0, 128), (128, 256), (256, 300)]               # D=300 contraction chunks
KH = [(0, 128), (128, 256), (256, 384), (384, 512)]   # H=512 contraction chunks
KD1 = [(0, 128), (128, 256), (256, 301)]              # D+1 (wadj+badj row)
MS = [(0, 128), (128, 256), (256, 384), (384, 512)]   # output chunks

# ---- scale constants (see emulate.py for the validated algebra) ----
SW1 = 32.0    # W1 stationary scale (w1a32/w1b8/w1brag32/b1row32)
SE8 = 4.0     # edges8 = SE8*q*edges       (A-copy scale = q*SE8/SW1)
SWE = 64.0    # we8 = SWE*We               (B-exp scale = 1/(SWE*SE8))
SE2 = 16.0    # edges2_8 = SE2*a*edges     (recaq = SE2/(SE8*q*suma))
SWV = 64.0    # wv8 = SWV*Wv
CF = 16.0     # w2aq8/w2bq8 = CF*q*W2xT
CM1 = 8192.0  # m1t8 = CM1*M1T             (copy scale = CM1/(CF*SWV) = 8)
CM2 = 512.0   # m2t8 = CM2*M2T (CM2*SE2 == CM1 so the F psum scales match)
RECQ_CLAMP = 1e4

_MAXW = 1  # this walrus build allows a single semaphore wait per instruction


def _split_multi_waits(nc):
    """Walrus here rejects instructions with >1 sem wait; hoist extra waits
    onto same-engine NoOps inserted just before the instruction."""
    ctr = 0
    for fn in nc.m.functions:
        for bb in fn.blocks:
            new = []
            for inst in bb.instructions:
                si = inst.sync_info
                if si is not None:
                    waits = list(si.on_wait)
                    if len(waits) > _MAXW:
                        for i in range(0, len(waits) - _MAXW, _MAXW):
                            ctr += 1
                            nop = mybir.InstNoOp(name=f"wsplit-{ctr}")
                            nop.engine = inst.engine
                            nop.sync_info = mybir.SyncInfo(
                                on_wait=waits[i : i + _MAXW], on_update=[]
                            )
                            new.append(nop)
                        si.on_wait = waits[len(waits) - _MAXW :]
                new.append(inst)
            bb.instructions = new
    return ctr


def _patch_ldw_dedupe():
    """The bass pipeline splits every matmul into Ldweights + Matmult.
    Consecutive matmuls that share the stationary operand then reload the
    same weights. Drop the redundant Ldweights at the BIR-JSON level
    (walrus's own --enable-ldw-opt rejects explicit Ldweights)."""
    import orjson

    import concourse.bass2jax as b2j
    import concourse.bass_utils as bu

    if getattr(bu, "_ldw_dedupe_patched", False):
        return
    orig = bu.compile_bir_kernel

    def _dedupe(bir_json):
        d = orjson.loads(bir_json)
        removed = 0
        nopctr = 0
        for fn in d.get("functions", []):
            stack = list(fn.get("blocks", []))
            while stack:
                blk = stack.pop()
                stack.extend(blk.get("blocks", []))
                insts = blk.get("instructions", [])
                out = []
                last_key = None
                for i in insts:
                    op = i.get("opcode")
                    if op == "Ldweights":
                        key = orjson.dumps(
                            [
                                i.get("ins"),
                                i.get("perf_mode"),
                                i.get("tile_position"),
                                i.get("tile_size"),
                                i.get("is_transpose"),
                            ]
                        )
                        si = i.get("sync_info") or {}
                        if key == last_key and not si.get("on_update"):
                            w = si.get("on_wait") or []
                            if w:
                                nopctr += 1
                                out.append(
                                    {
                                        "name": f"ldwkeep-{nopctr}",
                                        "opcode": "NoOp",
                                        "engine": i.get("engine", "PE"),
                                        "ins": [],
                                        "outs": [],
                                        "sync_info": {
                                            "on_wait": w,
                                            "on_update": [],
                                        },
                                    }
                                )
                            removed += 1
                            continue
                        last_key = key
                    elif op == "Matmult":
                        if i.get("is_transpose") or i.get("ldweights"):
                            last_key = None
                    out.append(i)
                blk["instructions"] = out
        if os.environ.get("KERNEL_DEBUG"):
            print(f"ldw dedupe: removed {removed}", file=sys.stderr)
        return orjson.dumps(d)

    def compile_bir_kernel(bir_json, tmpdir, neff_name="file.neff"):
        try:
            bir_json = _dedupe(bir_json)
        except Exception as e:  # pragma: no cover - safety net
            print(f"ldw dedupe skipped: {e}", file=sys.stderr)
        return orig(bir_json, tmpdir, neff_name=neff_name)

    bu.compile_bir_kernel = compile_bir_kernel
    b2j.compile_bir_kernel = compile_bir_kernel
    bu._ldw_dedupe_patched = True


def _install_ntff_hook():
    """Provide antenv.axon_hooks (missing in this image) so that
    run_bass_kernel_spmd(trace=True) can profile via libaxon_pjrt."""
    if "antenv.axon_hooks" in sys.modules:
        return

    def _mk(so_path):
        try:
            lib = ctypes.CDLL(so_path)
        except OSError:
            return None
        if not hasattr(lib, "axon_start_nrt_profile"):
            return None
        lib.axon_start_nrt_profile.argtypes = [
            ctypes.POINTER(ctypes.c_int64),
            ctypes.c_size_t,
        ]
        lib.axon_start_nrt_profile.restype = ctypes.c_int64
        lib.axon_stop_nrt_profile.argtypes = [ctypes.c_char_p]
        lib.axon_stop_nrt_profile.restype = ctypes.c_int64

        @contextlib.contextmanager
        def _hook(output_dir, device_ids):
            import jax

            jax.devices()
            if device_ids:
                ids = (ctypes.c_int64 * len(device_ids))(*device_ids)
                rc = lib.axon_start_nrt_profile(ids, len(device_ids))
            else:
                rc = lib.axon_start_nrt_profile(None, 0)
            if rc != 0:
                raise RuntimeError(f"axon_start_nrt_profile rc={rc}")
            try:
                yield
            finally:
                n = lib.axon_stop_nrt_profile(str(output_dir).encode())
                print(f"ntff profile: {n} file(s) -> {output_dir}", file=sys.stderr)

        return _hook

    hook = _mk("/opt/axon/libaxon_pjrt.so")
    mod = types.ModuleType("antenv.axon_hooks")
    mod.get_axon_ntff_profile_hook = lambda: hook
    try:
        import antenv

        antenv.axon_hooks = mod
    except ImportError:
        pass
    sys.modules["antenv.axon_hooks"] = mod

    import concourse.bass_utils as bass_utils

    bass_utils.upload_artifacts = lambda tmpdir: f"local://{tmpdir}"


def _re3(ap):
    """[128, n*E] -> [128, n, E] view."""
    return ap.rearrange("p (n e) -> p n e", e=E)


def build_program():
    nc = bass.Bass()

    # --- per-group data ---
    adjT = nc.declare_dram_parameter("adjT", [G, D + 1, TOK], BF, isOutput=False)
    adj8_d = nc.declare_dram_parameter("adj8", [G, 128, 2 * TOK], F8, isOutput=False)
    onT = nc.declare_dram_parameter("onT", [G, D, N], BF, isOutput=False)
    quesT = nc.declare_dram_parameter("quesT", [G, 128, 4], BF, isOutput=False)
    # --- static weights ---
    w1a32_d = nc.declare_dram_parameter("w1a32", [D, H], BF, isOutput=False)
    w1b8_d = nc.declare_dram_parameter("w1b8", [128, 2 * H], F8, isOutput=False)
    w1brag32_d = nc.declare_dram_parameter("w1brag32", [DRAG, H], BF, isOutput=False)
    b1row32_d = nc.declare_dram_parameter("b1row32", [1, H], BF, isOutput=False)
    smat_d = nc.declare_dram_parameter("smat", [N + 1, TOK], BF, isOutput=False)
    wq_d = nc.declare_dram_parameter("wq", [H, H], BF, isOutput=False)
    we8_d = nc.declare_dram_parameter("we8", [128, 4 * H], F8, isOutput=False)
    wv8_d = nc.declare_dram_parameter("wv8", [128, 4 * H], F8, isOutput=False)
    w2aT_d = nc.declare_dram_parameter("w2aT", [128, 4 * D], BF, isOutput=False)
    w2bT_d = nc.declare_dram_parameter("w2bT", [128, 4 * H], BF, isOutput=False)
    wadjx_d = nc.declare_dram_parameter("wadjx", [D + 1, H], BF, isOutput=False)
    # biases packed [128, 4] (column j = channels j*128..j*128+127)
    bq_d = nc.declare_dram_parameter("bq", [128, 4], F32, isOutput=False)
    be_d = nc.declare_dram_parameter("be", [128, 4], F32, isOutput=False)
    bv_d = nc.declare_dram_parameter("bv", [128, 4], F32, isOutput=False)
    b2c_d = nc.declare_dram_parameter("b2c", [128, 4], BF, isOutput=False)

    outT = nc.declare_dram_parameter("outT", [G, 4, 128, N], F32, isOutput=True)

    def tsl(t):
        return slice(t * T, (t + 1) * T)

    ctx0 = nc.allow_low_precision("softmax sums kept in bf16 deliberately")
    ctx0.__enter__()
    with tile.TileContext(nc) as tc, contextlib.ExitStack() as ctx:
        wpool = ctx.enter_context(tc.tile_pool(name="weights", bufs=1))
        gpool = ctx.enter_context(tc.tile_pool(name="group", bufs=2))
        gpool3 = ctx.enter_context(tc.tile_pool(name="group3", bufs=3))
        spool = ctx.enter_context(tc.tile_pool(name="small", bufs=2))
        spool3 = ctx.enter_context(tc.tile_pool(name="small3", bufs=3))
        ppool = ctx.enter_context(tc.tile_pool(name="pergroup", bufs=G))
        pspool = ctx.enter_context(tc.tile_pool(name="ps", bufs=8, space="PSUM"))

        # PE warmup: keep the HAM clock-gate at 8/8 through the startup
        # DMA wait so the first real matmuls run at 2.4 GHz.
        wu_sb = wpool.tile([128, 512], BF, tag="wu", name="wu")
        nc.vector.memset(wu_sb[:], 0.0)
        wu_ps = pspool.tile([128, T], F32, tag="ps", name="wups")
        for i in range(85):
            nc.tensor.matmul(
                wu_ps[:], wu_sb[:, :128], wu_sb[:, :T], start=True, stop=True
            )

        def load_w(dram, shape, dt_, name):
            t_ = wpool.tile(shape, dt_, tag=name, name=name)
            nc.scalar.dma_start(out=t_[:], in_=dram[:, :])
            return t_

        def load_w_chunks(dram, chunks, name):
            tiles = []
            for ki, (k0, k1) in enumerate(chunks):
                t_ = wpool.tile(
                    [k1 - k0, H], BF, tag=f"{name}{ki}", name=f"{name}{ki}"
                )
                nc.scalar.dma_start(out=t_[:], in_=dram[k0:k1, :])
                tiles.append(t_)
            return tiles

        w1a32_sb = load_w_chunks(w1a32_d, KD, "w1a32")
        wq_sb = load_w_chunks(wq_d, KH, "wq")
        wadjx_sb = load_w_chunks(wadjx_d, KD1, "wadjx")
        w1b8_sb = load_w(w1b8_d, [128, 2, H], F8, "w1b8")
        we8_sb = load_w(we8_d, [128, 4, H], F8, "we8")
        wv8_sb = load_w(wv8_d, [128, 4, H], F8, "wv8")
        w2aT_sb = load_w(w2aT_d, [128, 4, D], BF, "w2aT")
        w2bT_sb = load_w(w2bT_d, [128, 4, H], BF, "w2bT")
        bq_sb = load_w(bq_d, [128, 4], F32, "bq")
        be_sb = load_w(be_d, [128, 4], F32, "be")
        bv_sb = load_w(bv_d, [128, 4], F32, "bv")
        b2c_sb = load_w(b2c_d, [128, 4], BF, "b2c")

        def pre_dma_small(g):
            """Prologue DMAs: ques/on + the static rows of w1x."""
            st = {}
            ques_sb = spool.tile([128, 4], BF, tag="ques", name=f"ques_{g}")
            nc.sync.dma_start(out=ques_sb[:], in_=quesT[g, :, :])
            on_sb = []
            for ki, (k0, k1) in enumerate(KD):
                t_ = spool.tile(
                    [k1 - k0, N], BF, tag=f"on{ki}", name=f"on{ki}_{g}"
                )
                nc.sync.dma_start(out=t_[:], in_=onT[g, k0:k1, :])
                on_sb.append(t_)
            w1x_sb = ppool.tile([KX, H], BF, tag="w1x", name=f"w1x_{g}")
            nc.sync.dma_start(out=w1x_sb[N : N + 1, :], in_=b1row32_d[:, :])
            nc.sync.dma_start(out=w1x_sb[N + 1 :, :], in_=w1brag32_d[:, :])
            st["ques"] = ques_sb
            st["on"] = on_sb
            st["w1x"] = w1x_sb
            return st

        def pre_dma_big(g, st):
            """Per-iteration DMAs: adjacency tensors."""
            adjx_sb = gpool3.tile([KX, TOK], BF, tag="adjx", name=f"adjx_{g}")
            nc.sync.dma_start(out=adjx_sb[: N + 1, :], in_=smat_d[:, :])
            nc.sync.dma_start(out=adjx_sb[N + 1 :, :], in_=adjT[g, 256:D, :])
            adj8_sb = gpool3.tile([128, 2, TOK], F8, tag="adj8", name=f"adj8_{g}")
            nc.sync.dma_start(out=adj8_sb[:], in_=adj8_d[g, :, :])
            adjc = []
            for ki, (k0, k1) in enumerate(KD1):
                t_ = gpool3.tile(
                    [k1 - k0, TOK], BF, tag=f"adj{ki}", name=f"adj{ki}_{g}"
                )
                nc.sync.dma_start(out=t_[:], in_=adjT[g, k0:k1, :])
                adjc.append(t_)
            st["adjx"] = adjx_sb
            st["adj8"] = adj8_sb
            st["adjc"] = adjc

        def pre_early(g, st):
            """q chain + ontT + ACT weight folds (no M matmuls yet)."""
            ques_sb = st["ques"]
            on_sb = st["on"]
            w1x_sb = st["w1x"]

            # q = Wq @ ques + bq  (f32, kept for copy scales only)
            q_ps = pspool.tile([128, 4], F32, tag="ps", name=f"qps_{g}")
            for m, (m0, m1) in enumerate(MS):
                for k in range(4):
                    nc.tensor.matmul(
                        q_ps[:, m : m + 1],
                        wq_sb[k][:, m0:m1],
                        ques_sb[:, k : k + 1],
                        start=(k == 0),
                        stop=(k == 3),
                    )
            q_sb = spool.tile([128, 4], F32, tag="q", name=f"q_{g}")
            for m in range(4):
                nc.scalar.activation(
                    out=q_sb[:, m : m + 1],
                    in_=q_ps[:, m : m + 1],
                    func=ACTF.Identity,
                    bias=bq_sb[:, m : m + 1],
                )
            qA_sb = ppool.tile([128, 4], F32, tag="qA", name=f"qA_{g}")
            nc.scalar.mul(qA_sb[:], q_sb[:], SE8 / SW1)
            qC_sb = spool.tile([128, 4], F32, tag="qC", name=f"qC_{g}")
            nc.scalar.mul(qC_sb[:], q_sb[:], CF)
            rq_sb = spool.tile([128, 4], F32, tag="rq", name=f"rq_{g}")
            nc.vector.reciprocal(rq_sb[:], q_sb[:])
            recq_sb = spool.tile([128, 4], F32, tag="recq", name=f"recq_{g}")
            nc.vector.tensor_scalar(
                out=recq_sb[:],
                in0=rq_sb[:],
                scalar1=RECQ_CLAMP,
                scalar2=-RECQ_CLAMP,
                op0=ALU.min,
                op1=ALU.max,
            )
            # recqf = recq*(SE2/SE8): folded scale for the softmax-a chain
            recqf_sb = ppool.tile([128, 4], F32, tag="recqf", name=f"recqf_{g}")
            nc.scalar.mul(recqf_sb[:], recq_sb[:], SE2 / SE8)
            st["qA"] = qA_sb
            st["qC"] = qC_sb
            st["recqf"] = recqf_sb

            # transposed on-term: ontT[n, c] = SW1 * sum_f on[f, n] W1a[f, c]
            ontT_ps = pspool.tile([N, H], F32, tag="ps", name=f"ontTps_{g}")
            for ki in range(3):
                nc.tensor.matmul(
                    ontT_ps[:],
                    on_sb[ki][:],
                    w1a32_sb[ki][:],
                    start=(ki == 0),
                    stop=(ki == 2),
                )
            nc.scalar.copy(out=w1x_sb[:N, :], in_=ontT_ps[:])

            # folds: w2aq8 = CF*q (.) w2aT ; w2bq8 = CF*q (.) w2bT   (ACT)
            w2aq8 = ppool.tile([128, 4, H], F8, tag="w2aq8", name=f"w2aq8_{g}")
            w2bq8 = spool3.tile([128, 4, H], F8, tag="w2bq8", name=f"w2bq8_{g}")
            for k in range(4):
                nc.scalar.activation(
                    out=w2aq8[:, k, :D], in_=w2aT_sb[:, k, :],
                    func=ACTF.Copy, scale=qC_sb[:, k : k + 1],
                )
                nc.scalar.activation(
                    out=w2bq8[:, k, :], in_=w2bT_sb[:, k, :],
                    func=ACTF.Copy, scale=qC_sb[:, k : k + 1],
                )
            st["w2aq8"] = w2aq8
            st["w2bq8"] = w2bq8

        def pre_late(g, st):
            """M1/M2 fold matmuls + fp8 copies + b2 bias fold."""
            qC_sb = st["qC"]
            w2aq8 = st["w2aq8"]
            w2bq8 = st["w2bq8"]

            m1t8 = ppool.tile([128, 2, H], F8, tag="m1t8", name=f"m1t8_{g}")
            m1trag = ppool.tile([DRAG, H], BF, tag="m1trag", name=f"m1trag_{g}")
            for dc, (d0, d1) in enumerate(KD):
                psM = pspool.tile([d1 - d0, H], F32, tag="ps", name=f"m1ps_{g}_{dc}")
                for i in range(2):
                    nc.tensor.matmul(
                        psM[:],
                        w2aq8[:, 2 * i : 2 * i + 2, d0:d1],
                        wv8_sb[:, 2 * i : 2 * i + 2, :],
                        start=(i == 0),
                        stop=(i == 1),
                        perf_mode=DR,
                    )
                if dc < 2:
                    nc.vector.tensor_scalar_mul(
                        m1t8[:, dc, :], psM[:], CM1 / (CF * SWV)
                    )
                else:
                    nc.vector.tensor_scalar_mul(
                        m1trag[:, :], psM[:], CM1 / (CF * SWV)
                    )
            m2t8a = ppool.tile([128, 2, H], F8, tag="m2t8a", name=f"m2t8a_{g}")
            m2t8b = ppool.tile([128, 2, H], F8, tag="m2t8b", name=f"m2t8b_{g}")
            for kc, (k0, k1) in enumerate(KH):
                psM = pspool.tile([128, H], F32, tag="ps", name=f"m2ps_{g}_{kc}")
                for i in range(2):
                    nc.tensor.matmul(
                        psM[:],
                        w2bq8[:, 2 * i : 2 * i + 2, k0:k1],
                        wv8_sb[:, 2 * i : 2 * i + 2, :],
                        start=(i == 0),
                        stop=(i == 1),
                        perf_mode=DR,
                    )
                dst = m2t8a if kc < 2 else m2t8b
                nc.vector.tensor_scalar_mul(
                    dst[:, kc % 2, :], psM[:], CM2 / (CF * SWV)
                )
            st["m1t8"] = m1t8
            st["m1trag"] = m1trag
            st["m2t8a"] = m2t8a
            st["m2t8b"] = m2t8b

            # b2 correction folded into the F-exp bias:
            # bvx = bv + Wv @ (q*b2) = bv + wv8^T @ qb2_8 / (SWV*CF)
            qb2_8 = spool.tile([128, 4], F8, tag="qb2", name=f"qb2_{g}")
            nc.vector.tensor_tensor(
                out=qb2_8[:], in0=qC_sb[:], in1=b2c_sb[:], op=ALU.mult
            )
            qb2_ps = pspool.tile([128, 4], F32, tag="ps", name=f"qb2ps_{g}")
            for m, (m0, m1) in enumerate(MS):
                for k in range(4):
                    nc.tensor.matmul(
                        qb2_ps[:, m : m + 1],
                        wv8_sb[:, k, m0:m1],
                        qb2_8[:, k : k + 1],
                        start=(k == 0),
                        stop=(k == 3),
                    )
            bvx_sb = ppool.tile([128, 4], F32, tag="bvx", name=f"bvx_{g}")
            for m in range(4):
                nc.scalar.activation(
                    out=bvx_sb[:, m : m + 1],
                    in_=qb2_ps[:, m : m + 1],
                    func=ACTF.Identity,
                    scale=1.0 / (SWV * CF),
                    bias=bv_sb[:, m : m + 1],
                )
            st["bvx"] = bvx_sb

        def emit_A(g, st, ms):
            """Stage A (edges psum + fp8 copy) for m-chunks in ms."""
            adj8_sb = st["adj8"]
            w1x_sb = st["w1x"]
            adjx_sb = st["adjx"]
            qA_sb = st["qA"]
            if "edges8" not in st:
                st["edges8"] = gpool.tile(
                    [128, 4, TOK], F8, tag="edges8", name=f"edges8_{g}"
                )
                st["expa"] = [
                    gpool.tile([128, TOK], BF, tag=f"expa{m}", name=f"expa{m}_{g}")
                    for m in range(4)
                ]
                st["e2"] = gpool.tile([128, 4, TOK], F8, tag="e2", name=f"e2_{g}")
            edges8 = st["edges8"]

            for m in ms:
                m0, m1 = MS[m]
                eps = [
                    pspool.tile([128, T], F32, tag="ps", name=f"eps_{g}_{m}_{t}")
                    for t in range(NT)
                ]
                for t in range(NT):
                    nc.tensor.matmul(
                        eps[t][:],
                        w1b8_sb[:, :, m0:m1],
                        adj8_sb[:, :, tsl(t)],
                        start=True,
                        stop=False,
                        perf_mode=DR,
                    )
                for t in range(NT):
                    nc.tensor.matmul(
                        eps[t][:],
                        w1x_sb[:, m0:m1],
                        adjx_sb[:, tsl(t)],
                        start=False,
                        stop=True,
                    )
                # A-copy (ACT): edges8 = psum * (q*SE8/SW1), fp8 out
                for t in range(NT):
                    nc.scalar.activation(
                        out=edges8[:, m, tsl(t)],
                        in_=eps[t][:],
                        func=ACTF.Copy,
                        scale=qA_sb[:, m : m + 1],
                    )

        def emit_B(g, st):
            """Stage B: expa = exp(psum/(SWE*SE8) + be)."""
            edges8 = st["edges8"]
            expa_sb = st["expa"]
            for m, (m0, m1) in enumerate(MS):
                lps = [
                    pspool.tile([128, T], F32, tag="ps", name=f"lps_{g}_{m}_{t}")
                    for t in range(NT)
                ]
                for i in range(2):
                    for t in range(NT):
                        nc.tensor.matmul(
                            lps[t][:],
                            we8_sb[:, 2 * i : 2 * i + 2, m0:m1],
                            edges8[:, 2 * i : 2 * i + 2, tsl(t)],
                            start=(i == 0),
                            stop=(i == 1),
                            perf_mode=DR,
                        )
                for t in range(NT):
                    nc.scalar.activation(
                        out=expa_sb[m][:, tsl(t)],
                        in_=lps[t][:],
                        func=ACTF.Exp,
                        scale=1.0 / (SWE * SE8),
                        bias=be_sb[:, m : m + 1],
                    )

        def emit_CD(g, st):
            """softmax-a chain + edges2 per m-chunk."""
            edges8 = st["edges8"]
            expa_sb = st["expa"]
            e2_8 = st["e2"]
            recqf_sb = st["recqf"]
            for m in range(4):
                suma = spool.tile([128, N], F32, tag=f"suma{m}", name=f"suma{m}_{g}")
                nc.vector.tensor_reduce(
                    suma[:], _re3(expa_sb[m][:]), axis=AX.X, op=ALU.add
                )
                rs = spool.tile([128, N], F32, tag=f"rs{m}", name=f"rs{m}_{g}")
                nc.vector.reciprocal(rs[:], suma[:])
                recaq = spool.tile(
                    [128, N], F32, tag=f"recaq{m}", name=f"recaq{m}_{g}"
                )
                nc.vector.tensor_scalar_mul(recaq[:], rs[:], recqf_sb[:, m : m + 1])
                # aN = expa * recaq (broadcast over E) in place   (Pool)
                nc.gpsimd.tensor_tensor(
                    out=_re3(expa_sb[m][:]),
                    in0=_re3(expa_sb[m][:]),
                    in1=recaq[:, :, None].broadcast_to((128, N, E)),
                    op=ALU.mult,
                )
                # edges2_8 = aN * edges8 (Pool, fp8 out)
                nc.gpsimd.tensor_tensor(
                    out=e2_8[:, m, :],
                    in0=expa_sb[m][:],
                    in1=edges8[:, m, :],
                    op=ALU.mult,
                )

        def emit_F(g, st, ms):
            """Stage F' (logits_b via M1/M2) + expb for m-chunks in ms."""
            adjc = st["adjc"]
            adj8_sb = st["adj8"]
            e2_8 = st["e2"]
            m1t8 = st["m1t8"]
            m1trag = st["m1trag"]
            m2t8a = st["m2t8a"]
            m2t8b = st["m2t8b"]
            bvx_sb = st["bvx"]
            if "expb" not in st:
                st["expb"] = [
                    gpool.tile([128, TOK], BF, tag=f"expb{m}", name=f"expb{m}_{g}")
                    for m in range(4)
                ]
            expb_sb = st["expb"]

            for m in ms:
                m0, m1 = MS[m]
                bps = [
                    pspool.tile([128, T], F32, tag="ps", name=f"bps_{g}_{m}_{t}")
                    for t in range(NT)
                ]
                for t in range(NT):
                    nc.tensor.matmul(
                        bps[t][:],
                        m1t8[:, :, m0:m1],
                        adj8_sb[:, :, tsl(t)],
                        start=True,
                        stop=False,
                        perf_mode=DR,
                    )
                for t in range(NT):
                    nc.tensor.matmul(
                        bps[t][:],
                        m1trag[:, m0:m1],
                        adjc[2][:DRAG, tsl(t)],
                        start=False,
                        stop=False,
                    )
                for t in range(NT):
                    nc.tensor.matmul(
                        bps[t][:],
                        m2t8a[:, :, m0:m1],
                        e2_8[:, 0:2, tsl(t)],
                        start=False,
                        stop=False,
                        perf_mode=DR,
                    )
                for t in range(NT):
                    nc.tensor.matmul(
                        bps[t][:],
                        m2t8b[:, :, m0:m1],
                        e2_8[:, 2:4, tsl(t)],
                        start=False,
                        stop=True,
                        perf_mode=DR,
                    )
                for t in range(NT):
                    nc.scalar.activation(
                        out=expb_sb[m][:, tsl(t)],
                        in_=bps[t][:],
                        func=ACTF.Exp,
                        scale=1.0 / CM1,
                        bias=bvx_sb[:, m : m + 1],
                    )

        def emit_G(g, st):
            expb_sb = st["expb"]
            recb_sb = []
            for m in range(4):
                sumb = spool.tile([128, N], F32, tag=f"sumb{m}", name=f"sumb{m}_{g}")
                nc.vector.tensor_reduce(
                    sumb[:], _re3(expb_sb[m][:]), axis=AX.X, op=ALU.add
                )
                recb = spool.tile([128, N], F32, tag=f"recb{m}", name=f"recb{m}_{g}")
                nc.vector.reciprocal(recb[:], sumb[:])
                recb_sb.append(recb)
            st["recb"] = recb_sb

        def emit_H(g, st, ms):
            """Stage H (bf16, badj via ones-row) for m-chunks in ms."""
            adjc = st["adjc"]
            expb_sb = st["expb"]
            for m in ms:
                m0, m1 = MS[m]
                aps = [
                    pspool.tile([128, T], F32, tag="ps", name=f"aps_{g}_{m}_{t}")
                    for t in range(NT)
                ]
                for ki in range(3):
                    for t in range(NT):
                        nc.tensor.matmul(
                            aps[t][:],
                            wadjx_sb[ki][:, m0:m1],
                            adjc[ki][:, tsl(t)],
                            start=(ki == 0),
                            stop=(ki == 2),
                        )
                # pre = psum * expb  (DVE - GPSIMD cannot access PSUM)
                for t in range(NT):
                    nc.vector.tensor_tensor(
                        out=expb_sb[m][:, tsl(t)],
                        in0=aps[t][:],
                        in1=expb_sb[m][:, tsl(t)],
                        op=ALU.mult,
                    )

        def emit_I(g, st):
            expb_sb = st["expb"]
            recb_sb = st["recb"]
            for m in range(4):
                s_sb = spool.tile([128, N], F32, tag=f"s{m}", name=f"s{m}_{g}")
                nc.vector.tensor_reduce(
                    s_sb[:], _re3(expb_sb[m][:]), axis=AX.X, op=ALU.add
                )
                o_sb = spool.tile([128, N], F32, tag=f"o{m}", name=f"o{m}_{g}")
                nc.vector.tensor_tensor(
                    out=o_sb[:], in0=s_sb[:], in1=recb_sb[m][:], op=ALU.mult
                )
                nc.sync.dma_start(out=outT[g, m, :, :], in_=o_sb[:])

        # Prologue: all per-group q/ontT/fold/M compute up front (ques/on
        # are tiny and available immediately); the steady-state loop then
        # contains only A/B/CD/F/G/H/I with exactly 64 PSUM allocations per
        # iteration (8-bank aligned) and no serial preamble chains.
        states = {}
        states[0] = pre_dma_small(0)
        for g in range(G):
            if g + 1 < G:
                states[g + 1] = pre_dma_small(g + 1)
            pre_early(g, states[g])
        for g in range(G):
            pre_late(g, states[g])
        pre_dma_big(0, states[0])
        if G > 1:
            pre_dma_big(1, states[1])
        for g in range(G):
            st = states[g]
            stp = states.get(g - 1)
            emit_A(g, st, (0, 1))
            if g + 2 < G:
                pre_dma_big(g + 2, states[g + 2])
            if stp:
                emit_F(g - 1, stp, (0, 1))
            emit_A(g, st, (2, 3))
            if stp:
                emit_F(g - 1, stp, (2, 3))
                emit_G(g - 1, stp)
            emit_B(g, st)
            if stp:
                emit_H(g - 1, stp, (0, 1))
            emit_CD(g, st)
            if stp:
                emit_H(g - 1, stp, (2, 3))
                emit_I(g - 1, stp)
                del states[g - 1]
        st = states[G - 1]
        emit_F(G - 1, st, (0, 1))
        emit_F(G - 1, st, (2, 3))
        emit_G(G - 1, st)
        emit_H(G - 1, st, (0, 1))
        emit_H(G - 1, st, (2, 3))
        emit_I(G - 1, st)

    ctx0.__exit__(None, None, None)
    nsplit = _split_multi_waits(nc)
    if os.environ.get("KERNEL_DEBUG"):
        print(f"split_multi_waits: {nsplit} nops inserted", file=sys.stderr)
    return nc


def _pack_bias(b, dt=np.float32):
    # [H] -> [128, 4]: column j = channels j*128..(j+1)*128
    return np.ascontiguousarray(
        np.asarray(b, np.float32).reshape(4, 128).T.astype(dt)
    )


def _bf(x):
    return np.ascontiguousarray(np.asarray(x, np.float32).astype(ml_dtypes.bfloat16))


def _f8(x):
    return np.ascontiguousarray(
        np.asarray(x, np.float32).astype(ml_dtypes.float8_e4m3fn)
    )


def _pack_planes(x, nplanes):
    """[nplanes*128, F] -> [128, nplanes*F] (plane-major free dim)."""
    x = np.asarray(x)
    K, F = x.shape
    assert K == nplanes * 128
    return np.ascontiguousarray(
        x.reshape(nplanes, 128, F).transpose(1, 0, 2).reshape(128, nplanes * F)
    )


def _smat():
    """[N+1, TOK] node->token selection matrix (+ ones row for the b1 bias)."""
    s = np.zeros((N + 1, TOK), np.float32)
    for n in range(N):
        s[n, n * E : (n + 1) * E] = 1.0
    s[N, :] = 1.0
    return _bf(s)


def prepare_inputs(ques_embed, adj_list, original_nodes,
                   w1_w, w1_b, wq_w, wq_b, we_w, we_b,
                   w2_w, w2_b, wv_w, wv_b, wadj_w, wadj_b):
    """Host-side layout prep. Returns a list of per-core input maps."""
    adjTf = np.asarray(adj_list, np.float32).reshape(BR, TOK, D).transpose(0, 2, 1)
    # adjT with a trailing ones row (badj fold for stage H)
    adjT = np.empty((BR, D + 1, TOK), ml_dtypes.bfloat16)
    adjT[:, :D, :] = adjTf.astype(ml_dtypes.bfloat16)
    adjT[:, D, :] = np.asarray(1.0, ml_dtypes.bfloat16)
    adj8 = np.ascontiguousarray(
        adjTf[:, :256, :].astype(ml_dtypes.float8_e4m3fn)
        .reshape(BR, 2, 128, TOK).transpose(0, 2, 1, 3).reshape(BR, 128, 2 * TOK)
    )
    onT = _bf(
        np.asarray(original_nodes, np.float32).reshape(BR, N, D).transpose(0, 2, 1)
    )
    quesT = _bf(
        np.asarray(ques_embed, np.float32).reshape(BR, 4, 128).transpose(0, 2, 1)
    )

    w1 = np.asarray(w1_w, np.float32)
    w1a = w1[:, :D].T          # [D, H]
    w1b = w1[:, D:].T          # [D, H]
    w2 = np.asarray(w2_w, np.float32)
    wadjT = np.asarray(wadj_w, np.float32).T   # [D, H]
    wadjx = np.concatenate(
        [wadjT, np.asarray(wadj_b, np.float32)[None, :]], axis=0
    )

    w = {
        "w1a32": _bf(SW1 * w1a),
        "w1b8": _pack_planes(_f8(SW1 * w1b[:256]), 2),
        "w1brag32": _bf(SW1 * w1b[256:]),
        "b1row32": _bf(SW1 * np.asarray(w1_b, np.float32).reshape(1, H)),
        "smat": _smat(),
        "wq": _bf(np.asarray(wq_w).T),
        "we8": _pack_planes(_f8(SWE * np.asarray(we_w, np.float32).T), 4),
        "wv8": _pack_planes(_f8(SWV * np.asarray(wv_w, np.float32).T), 4),
        "w2aT": _pack_planes(_bf(w2[:, :D]), 4),
        "w2bT": _pack_planes(_bf(w2[:, D:]), 4),
        "wadjx": _bf(wadjx),
        "bq": _pack_bias(wq_b),
        "be": _pack_bias(we_b),
        "bv": _pack_bias(wv_b),
        "b2c": _pack_bias(w2_b, ml_dtypes.bfloat16),
    }

    in_maps = []
    for c in range(NCORES):
        sl = slice(c * G, (c + 1) * G)
        m = dict(w)
        m["adjT"] = np.ascontiguousarray(adjT[sl])
        m["adj8"] = np.ascontiguousarray(adj8[sl])
        m["onT"] = np.ascontiguousarray(onT[sl])
        m["quesT"] = np.ascontiguousarray(quesT[sl])
        in_maps.append(m)
    return in_maps


def run(in_maps, trace=False, tmpdir=None):
    _install_ntff_hook()
    if not os.environ.get("KERNEL_NO_LDW_DEDUPE"):
        _patch_ldw_dedupe()
    from concourse.bass_utils import run_bass_kernel_spmd

    nc = build_program()
    res = run_bass_kernel_spmd(
        nc,
        in_maps,
        core_ids=list(range(NCORES)),
        trace=trace,
        tmpdir=tmpdir,
    )
    return res


def gather_output(res):
    outT = np.stack([res.results[c]["outT"] for c in range(NCORES)])  # [8,5,4,128,N]
    outT = outT.reshape(BR, 4, 128, N).transpose(0, 3, 1, 2)          # [40,N,4,128]
    return np.ascontiguousarray(outT.reshape(B, R, N, H).astype(np.float32))


def kernel(ques_embed, adj_list, original_nodes,
           w1_w, w1_b, wq_w, wq_b, we_w, we_b,
           w2_w, w2_b, wv_w, wv_b, wadj_w, wadj_b,
           deg=None, batch_size=None, **_unused):
    in_maps = prepare_inputs(
        ques_embed, adj_list, original_nodes,
        w1_w, w1_b, wq_w, wq_b, we_w, we_b,
        w2_w, w2_b, wv_w, wv_b, wadj_w, wadj_b,
    )
    res = run(in_maps, trace=False)
    return gather_output(res)
